# revision 1
# baseline (speedup 1.0000x reference)
"""GemmaAttention (B=2, S=2048, D=2048, H=8, KV=1, HD=256) on 8 trn2 NeuronCores.

Sharding: DP=2 over batch x TP=4 over head-pairs. Core c handles batch c//4 and
heads {2*(c%4), 2*(c%4)+1}. Each core computes its partial o_proj output
(row-parallel Wo); the host sums the 4 partials per batch (the all-reduce is
folded into the host-side unshard).

Dataflow on each core (everything float32r on the PE at full rate):
  QT[dq,s]  = Wq_sl.T @ hT   (hT = hidden[b].T, host-transposed)
  KT[dk,s]  = Wk.T   @ hT
  V[s,dv]   = (hT chunks as lhsT) @ Wv
  RoPE applied to QT/KT in the psum->SBUF drain (DVE), with 1/sqrt(HD) folded
  into the exp's scale argument.
  scoresT[k,q] = KT_chunk.T @ QT  (per head)
  expT = ACT Exp(scoresT * 1/16) (+ causal staircase / external mask)
  outT[dv,q] += V_chunk.T @ expT ; denominators via DVE accumulation of expT
  plus a ones-vector matmul partition-reduce; normalize outT by 1/sum.
  out_partial[s,:] = outTn_chunk.T @ Wo_sl   -> DMA to DRAM.
"""

import numpy as np

import concourse.bass as bass
import concourse.tile as tile
import concourse.mybir as mybir
from concourse import bacc
from concourse.bass_utils import run_bass_kernel_spmd
from concourse._compat import with_exitstack  # noqa: F401

P = 128
B, S, D = 2, 2048, 2048
H, KV, HD = 8, 1, 256
ROPE_BASE = 10000.0
NEG_BIG = -1.0e30

HEADS_PER_CORE = 2
DQ = HEADS_PER_CORE * HD          # 512 q-dims per core
DCH = D // P                      # 16 contraction chunks
SBLK = 512                        # s-tile for projection rhs / q-tile
NSBLK = S // SBLK                 # 4
NKC = S // P                      # 16 key chunks
NQCH = DQ // P                    # 4 QT partition chunks
NKCH = HD // P                    # 2 KT partition chunks

F32 = mybir.dt.float32
F32R = mybir.dt.float32r
EXP = mybir.ActivationFunctionType.Exp

# exec time of the last traced run (set by run_spmd when tracing)
LAST_EXEC_TIME_NS = None

_BUILD_CACHE = {}


def _build(causal: bool):
    nc = bacc.Bacc()

    hT = nc.declare_dram_parameter("hT", [D, S], F32R, isOutput=False)
    wq = nc.declare_dram_parameter("wq", [D, DQ], F32R, isOutput=False)
    wk = nc.declare_dram_parameter("wk", [D, HD], F32R, isOutput=False)
    wv = nc.declare_dram_parameter("wv", [D, HD], F32R, isOutput=False)
    wo = nc.declare_dram_parameter("wo", [DQ, D], F32R, isOutput=False)
    cosT = nc.declare_dram_parameter("cosT", [HD, S], F32, isOutput=False)
    sinT = nc.declare_dram_parameter("sinT", [HD, S], F32, isOutput=False)
    ones = nc.declare_dram_parameter("ones", [P, P], F32R, isOutput=False)
    ident = nc.declare_dram_parameter("ident", [P, P], F32R, isOutput=False)
    if causal:
        stair = nc.declare_dram_parameter("stair", [P, 2 * SBLK], F32, isOutput=False)
    else:
        maskT = nc.declare_dram_parameter("emaskT", [S, S], F32, isOutput=False)
    outp = nc.declare_dram_parameter("out_partial", [S, D], F32, isOutput=True)

    from contextlib import ExitStack
    from collections import deque
    with tile.TileContext(nc) as tc, ExitStack() as ctx:
        # persistent pools
        pq = ctx.enter_context(tc.tile_pool(name="pq", bufs=1))
        QT = pq.tile([P, NQCH, S], F32R, name="QT")
        KT = pq.tile([P, NKCH, S], F32R, name="KT")
        VN = pq.tile([P, NKC, HD], F32R, name="VN")
        ONES = pq.tile([P, P], F32R, name="ONES")
        IDENT = pq.tile([P, P], F32R, name="IDENT")
        ONEC = ONES[:, 0:1]
        ONER = ONES[0:1, :]

        # ---- phase A+B: projections + RoPE ----------------------------
        with tc.tile_pool(name="pw", bufs=1) as pw, \
             tc.tile_pool(name="pht", bufs=16) as pht, \
             tc.tile_pool(name="pcs", bufs=2) as pcs, \
             tc.tile_pool(name="pvt", bufs=2) as pvt, \
             tc.tile_pool(name="ptmp", bufs=2) as ptmp, \
             tc.tile_pool(name="pjp", bufs=8, space="PSUM") as pp:
            WQ = pw.tile([P, DCH, DQ], F32R, name="WQ")
            WK = pw.tile([P, DCH, HD], F32R, name="WK")
            WV = pw.tile([P, DCH, HD], F32R, name="WV")

            for sb in range(NSBLK):
                ssl = slice(sb * SBLK, (sb + 1) * SBLK)
                psq = [pp.tile([P, SBLK], F32, name="pp") for _ in range(NQCH)]
                psk = [pp.tile([P, SBLK], F32, name="pp") for _ in range(NKCH)]
                psvt = [pp.tile([P, SBLK], F32, name="pp") for _ in range(2)]
                COSb = pcs.tile([P, NKCH, SBLK], F32, name="cosb")
                SINb = pcs.tile([P, NKCH, SBLK], F32, name="sinb")
                hts = []
                for c in range(DCH):
                    ht = pht.tile([P, SBLK], F32R, name="ht")
                    hts.append(ht)
                    nc.sync.dma_start(out=ht, in_=hT[c * P:(c + 1) * P, ssl])
                    if sb == 0:
                        # weight chunks stream just behind their first use
                        nc.sync.dma_start(out=WQ[:, c, :], in_=wq[c * P:(c + 1) * P, :])
                        nc.sync.dma_start(out=WK[:, c, :], in_=wk[c * P:(c + 1) * P, :])
                        nc.sync.dma_start(out=WV[:, c, :], in_=wv[c * P:(c + 1) * P, :])
                        if c == 0:
                            nc.sync.dma_start(out=ONES, in_=ones[:, :])
                            nc.sync.dma_start(out=IDENT, in_=ident[:, :])
                    if 2 <= c < 2 + NKCH:
                        nc.sync.dma_start(out=COSb[:, c - 2, :],
                                          in_=cosT[(c - 2) * P:(c - 1) * P, ssl])
                        nc.sync.dma_start(out=SINb[:, c - 2, :],
                                          in_=sinT[(c - 2) * P:(c - 1) * P, ssl])
                    for i in range(NQCH):
                        nc.tensor.matmul(psq[i], lhsT=WQ[:, c, i * P:(i + 1) * P],
                                         rhs=ht, start=(c == 0), stop=(c == DCH - 1))
                    for j in range(NKCH):
                        nc.tensor.matmul(psk[j], lhsT=WK[:, c, j * P:(j + 1) * P],
                                         rhs=ht, start=(c == 0), stop=(c == DCH - 1))
                    for j in range(2):
                        nc.tensor.matmul(psvt[j], lhsT=WV[:, c, j * P:(j + 1) * P],
                                         rhs=ht, start=(c == 0), stop=(c == DCH - 1))
                # RoPE drains (fused psum->SBUF)
                def rope_pair(p0, p1, out0, out1):
                    c0 = COSb[:, 0, :]; c1 = COSb[:, 1, :]
                    s0 = SINb[:, 0, :]; s1 = SINb[:, 1, :]
                    t1 = ptmp.tile([P, SBLK], F32, name="t")
                    t2 = ptmp.tile([P, SBLK], F32, name="t")
                    nc.vector.tensor_mul(t1, p0, c0)
                    nc.vector.tensor_mul(t2, p1, s0)
                    nc.vector.tensor_sub(out0, t1, t2)
                    t3 = ptmp.tile([P, SBLK], F32, name="t")
                    t4 = ptmp.tile([P, SBLK], F32, name="t")
                    nc.vector.tensor_mul(t3, p1, c1)
                    nc.vector.tensor_mul(t4, p0, s1)
                    nc.vector.tensor_add(out1, t3, t4)
                for h in range(HEADS_PER_CORE):
                    rope_pair(psq[2 * h], psq[2 * h + 1],
                              QT[:, 2 * h, ssl], QT[:, 2 * h + 1, ssl])
                rope_pair(psk[0], psk[1], KT[:, 0, ssl], KT[:, 1, ssl])
                # VT drain then PE-transpose into natural V layout
                vts = []
                for j in range(2):
                    vt = pvt.tile([P, SBLK], F32R, name="vt")
                    nc.vector.tensor_copy(vt, psvt[j])
                    vts.append(vt)
                for si in range(SBLK // P):
                    for j in range(2):
                        pst = pp.tile([P, P], F32R, name="pp")
                        nc.tensor.transpose(pst, vts[j][:, si * P:(si + 1) * P], IDENT)
                        nc.vector.tensor_copy(
                            VN[:, sb * (SBLK // P) + si, j * P:(j + 1) * P], pst)

        # ---- late persistent: o_proj weights + normalized outT --------
        patt = ctx.enter_context(tc.tile_pool(name="patt", bufs=1))
        WO = patt.tile([P, NQCH, D], F32R, name="WO")
        for c in range(NQCH):
            nc.sync.dma_start(out=WO[:, c, :], in_=wo[c * P:(c + 1) * P, :])
        OUTN = patt.tile([P, NQCH, S], F32R, name="OUTN")

        # ---- phase C+D: attention + interleaved o_proj ----------------
        with tc.tile_pool(name="pexp", bufs=6) as pexp, \
             tc.tile_pool(name="pacc", bufs=4) as pacc, \
             tc.tile_pool(name="pou", bufs=8) as pou, \
             tc.tile_pool(name="pmisc", bufs=2) as pmisc, \
             tc.tile_pool(name="pmask", bufs=4) as pmask, \
             tc.tile_pool(name="pfin", bufs=3) as pfin, \
             tc.tile_pool(name="ps_s", bufs=3, space="PSUM") as ps_s, \
             tc.tile_pool(name="ps_o", bufs=2, space="PSUM") as ps_o, \
             tc.tile_pool(name="ps_r", bufs=2, space="PSUM") as ps_r, \
             tc.tile_pool(name="ps_f", bufs=1, space="PSUM") as ps_f:
            if causal:
                STAIR = pq.tile([P, 2 * SBLK], F32, name="STAIR")
                nc.sync.dma_start(out=STAIR, in_=stair[:, :])

            def emit_norm(pend):
                ou, acc, h, qb = pend
                qsl = slice(qb * SBLK, (qb + 1) * SBLK)
                pssum = ps_r.tile([P, SBLK], F32, name="pr")
                nc.tensor.matmul(pssum[0:1, :], lhsT=ONEC, rhs=acc)
                rsb = pmisc.tile([1, SBLK], F32R, name="rsb")
                with nc.allow_low_precision("f32r output is f32-width"):
                    nc.vector.reciprocal(rsb, pssum[0:1, :])
                psb = ps_r.tile([P, SBLK], F32, name="pr")
                nc.tensor.matmul(psb, lhsT=ONER, rhs=rsb)
                rbc = pmisc.tile([P, SBLK], F32R, name="rbc")
                nc.scalar.copy(rbc, psb)
                for dvc in range(2):
                    nc.vector.tensor_mul(OUTN[:, 2 * h + dvc, qsl], ou[dvc], rbc)

            def emit_oproj_quarter(qb):
                for st in range(4 * qb, 4 * qb + 4):
                    stsl = slice(st * P, (st + 1) * P)
                    for nb in range(NSBLK):
                        psf = ps_f.tile([P, SBLK], F32, name="pf")
                        for dvc in range(NQCH):
                            nc.tensor.matmul(psf, lhsT=OUTN[:, dvc, stsl],
                                             rhs=WO[:, dvc, nb * SBLK:(nb + 1) * SBLK],
                                             start=(dvc == 0), stop=(dvc == NQCH - 1))
                        fsb = pfin.tile([P, SBLK], F32, name="fsb")
                        nc.scalar.copy(fsb, psf)
                        nc.sync.dma_start(out=outp[stsl, nb * SBLK:(nb + 1) * SBLK],
                                          in_=fsb)

            pending = deque()
            for h in range(HEADS_PER_CORE):
                for qb in range(NSBLK):
                    qsl = slice(qb * SBLK, (qb + 1) * SBLK)
                    klim = 4 * (qb + 1) if causal else NKC
                    pso = [ps_o.tile([P, SBLK], F32, name="po") for _ in range(2)]
                    acc = pacc.tile([P, SBLK], F32R, name="acc")
                    for kc0 in range(0, klim, 2):
                        kcs = [kc0, kc0 + 1]
                        exs = []
                        for kc in kcs:
                            pss = ps_s.tile([P, SBLK], F32, name="ps")
                            for c in range(NKCH):
                                nc.tensor.matmul(pss,
                                                 lhsT=KT[:, c, kc * P:(kc + 1) * P],
                                                 rhs=QT[:, 2 * h + c, qsl],
                                                 start=(c == 0), stop=(c == NKCH - 1))
                            ex = pexp.tile([P, SBLK], F32R, name="ex")
                            nc.scalar.activation(ex, pss, EXP, scale=1.0 / 16.0)
                            if causal and kc >= 4 * qb:
                                delta = 128 * kc - 512 * qb
                                nc.vector.tensor_mul(ex, ex,
                                                     STAIR[:, 512 - delta:1024 - delta])
                            if not causal:
                                mt = pmask.tile([P, SBLK], F32, name="mt")
                                nc.sync.dma_start(
                                    out=mt, in_=maskT[kc * P:(kc + 1) * P, qsl])
                                nc.vector.tensor_mul(ex, ex, mt)
                            exs.append(ex)
                        for kc, ex in zip(kcs, exs):
                            if kc == 0:
                                nc.vector.tensor_copy(acc, ex)
                            else:
                                nc.vector.tensor_add(acc, acc, ex)
                        for kc, ex in zip(kcs, exs):
                            for dvc in range(2):
                                nc.tensor.matmul(pso[dvc],
                                                 lhsT=VN[:, kc, dvc * P:(dvc + 1) * P],
                                                 rhs=ex, start=(kc == 0),
                                                 stop=(kc == klim - 1))
                    ou = [pou.tile([P, SBLK], F32R, name="ou") for _ in range(2)]
                    for dvc in range(2):
                        nc.vector.tensor_copy(ou[dvc], pso[dvc])
                    pending.append((ou, acc, h, qb))
                    if len(pending) > 2:
                        p = pending.popleft()
                        emit_norm(p)
                        if p[2] == 1:
                            emit_oproj_quarter(p[3])
            while pending:
                p = pending.popleft()
                emit_norm(p)
                if p[2] == 1:
                    emit_oproj_quarter(p[3])

    nc.finalize()
    return nc


def _get_nc(causal: bool):
    key = bool(causal)
    if key not in _BUILD_CACHE:
        _BUILD_CACHE[key] = _build(causal)
    return _BUILD_CACHE[key]


def _rope_tables(position_ids_b):
    # cosT/sinT: [HD, S] fp32, transposed layout for the [d, s] dataflow
    pos = np.asarray(position_ids_b, dtype=np.float64)
    inv = 1.0 / (ROPE_BASE ** (np.arange(0, HD, 2, dtype=np.float64) / HD))
    f = pos[:, None] * inv[None, :]            # [S, HD/2]
    emb = np.concatenate([f, f], axis=1)       # [S, HD]
    cosT = np.ascontiguousarray(np.cos(emb).T.astype(np.float32))
    sinT = np.ascontiguousarray(np.sin(emb).T.astype(np.float32))
    return cosT, sinT


def _is_causal(attention_mask):
    m = np.asarray(attention_mask)
    if m.shape != (B, 1, S, S):
        return False
    tri = np.tril(np.ones((S, S), dtype=bool))
    canon = np.where(tri, np.float32(0.0), np.float32(-1e9))
    return all(np.array_equal(m[b, 0], canon) for b in range(B))


_ONES_NP = np.ones((P, P), dtype=np.float32)
_IDENT_NP = np.eye(P, dtype=np.float32)


def _stair():
    # multiplicative staircase: stair01[p, j] = 1 if (j - 512) >= p else 0
    j = np.arange(2 * SBLK)[None, :] - SBLK
    p = np.arange(P)[:, None]
    return np.where(j >= p, np.float32(1.0), np.float32(0.0)).astype(np.float32)


def kernel(hidden_state, attention_mask, position_ids, Wq, Wk, Wv, Wo,
           _trace=False, _tmpdir=None):
    global LAST_EXEC_TIME_NS
    hidden_state = np.asarray(hidden_state, dtype=np.float32)
    Wq = np.asarray(Wq, dtype=np.float32)
    Wk = np.asarray(Wk, dtype=np.float32)
    Wv = np.asarray(Wv, dtype=np.float32)
    Wo = np.asarray(Wo, dtype=np.float32)

    causal = _is_causal(attention_mask)
    nc = _get_nc(causal)

    stair = _stair() if causal else None
    in_maps = []
    per_batch = {}
    for b in range(B):
        hTb = np.ascontiguousarray(hidden_state[b].T)          # [D, S]
        cosT, sinT = _rope_tables(position_ids[b])
        mb = None
        if not causal:
            mb = np.ascontiguousarray(
                np.exp(np.asarray(attention_mask, dtype=np.float64)[b, 0].T)
                .astype(np.float32))
        per_batch[b] = (hTb, cosT, sinT, mb)

    for core in range(8):
        b = core // 4
        hp = core % 4
        hTb, cosT, sinT, mb = per_batch[b]
        im = {
            "hT": hTb,
            "ones": _ONES_NP,
            "ident": _IDENT_NP,
            "wq": np.ascontiguousarray(Wq[:, hp * DQ:(hp + 1) * DQ]),
            "wk": Wk,
            "wv": Wv,
            "wo": np.ascontiguousarray(Wo[hp * DQ:(hp + 1) * DQ, :]),
            "cosT": cosT,
            "sinT": sinT,
        }
        if causal:
            im["stair"] = stair
        else:
            im["maskT16"] = mb
        in_maps.append(im)

    res = run_bass_kernel_spmd(nc, in_maps, core_ids=list(range(8)),
                               trace=_trace, tmpdir=_tmpdir)
    LAST_EXEC_TIME_NS = res.exec_time_ns

    out = np.empty((B, S, D), dtype=np.float32)
    for b in range(B):
        acc = res.results[4 * b]["out_partial"].astype(np.float32).copy()
        for hp in range(1, 4):
            acc += res.results[4 * b + hp]["out_partial"]
        out[b] = acc
    return out



# revision 5
# speedup vs baseline: 1.4356x; 1.4356x over previous
"""GemmaAttention (B=2, S=2048, D=2048, H=8, KV=1, HD=256) on 8 trn2 NeuronCores.

Sharding: DP=2 over batch x TP=4 over head-pairs. Core c handles batch c//4 and
heads {2*(c%4), 2*(c%4)+1}. Each core computes its partial o_proj output
(row-parallel Wo); the host sums the 4 partials per batch (the all-reduce is
folded into the host-side unshard).

All matmuls run in bf16 (fp32 PSUM accumulate): fp32r streams at ~half the
bf16 column rate on the PE (389ns vs 213ns per N=512 matmul), so bf16 nearly
doubles tensor-engine throughput. rel-err budget is 2e-2; bf16 lands ~3e-3.

Dataflow per core:
  phase A (projections, per 512-col s-block):
    QT[dq,s], KT[dk,s] accumulate c-major over 16 D-chunks (6 PSUM banks),
    drained via ACT copy (psum->bf16 SBUF, frees banks fast) then RoPE on DVE
    in bf16 (2x mode). V[s,dv] computed directly (lhsT = hT chunk) in 128-row
    chains on 2 more banks -- this PE work covers the RoPE drain latency so
    the PE never idles at s-block boundaries.
  phase C (attention, per (head, q-block) item, software-pipelined):
    scoresT[k,q] = KT_chunk.T @ QT with a skew-2 pipeline: S(i) issues two
    iterations ahead of DEN(i)/AV(i) so the ACT exp (+DVE causal staircase)
    latency is hidden behind PE work. Denominators accumulate on the PE
    (ONEC.T @ ex into a [1,512] PSUM row). Per-item normalization is deferred
    one item (reciprocal_approx_fast + ONES-broadcast matmul + DVE scale) and
    o_proj quarters are injected into the next item's PE stream.
"""

import numpy as np
import ml_dtypes

import concourse.bass as bass
import concourse.tile as tile
import concourse.mybir as mybir
from concourse import bacc
from concourse.bass_utils import run_bass_kernel_spmd

P = 128
B, S, D = 2, 2048, 2048
H, KV, HD = 8, 1, 256
ROPE_BASE = 10000.0

HEADS_PER_CORE = 2
DQ = HEADS_PER_CORE * HD          # 512 q-dims per core
DCH = D // P                      # 16 contraction chunks
SBLK = 512                        # s-tile for projection rhs / q-tile
NSBLK = S // SBLK                 # 4
NKC = S // P                      # 16 key chunks
NQCH = DQ // P                    # 4 QT partition chunks
NKCH = HD // P                    # 2 KT partition chunks

F32 = mybir.dt.float32
BF16 = mybir.dt.bfloat16
EXP = mybir.ActivationFunctionType.Exp

LAST_EXEC_TIME_NS = None

_BUILD_CACHE = {}


def _build(causal: bool):
    nc = bacc.Bacc()

    hT = nc.declare_dram_parameter("hT", [D, S], BF16, isOutput=False)
    wq = nc.declare_dram_parameter("wq", [D, DQ], BF16, isOutput=False)
    wk = nc.declare_dram_parameter("wk", [D, HD], BF16, isOutput=False)
    wv = nc.declare_dram_parameter("wv", [D, HD], BF16, isOutput=False)
    wo = nc.declare_dram_parameter("wo", [DQ, D], BF16, isOutput=False)
    cosT = nc.declare_dram_parameter("cosT", [HD, S], BF16, isOutput=False)
    sinT = nc.declare_dram_parameter("sinT", [HD, S], BF16, isOutput=False)
    onesb = nc.declare_dram_parameter("onesb", [P, P], BF16, isOutput=False)
    onesf = nc.declare_dram_parameter("onesf", [1, P], F32, isOutput=False)
    if causal:
        stair = nc.declare_dram_parameter("stair", [P, 2 * SBLK], BF16,
                                          isOutput=False)
    else:
        maskT = nc.declare_dram_parameter("emaskT", [S, S], BF16, isOutput=False)
    outp = nc.declare_dram_parameter("out_partial", [S, D], F32, isOutput=True)

    from contextlib import ExitStack
    with tile.TileContext(nc) as tc, ExitStack() as ctx:
        pq = ctx.enter_context(tc.tile_pool(name="pq", bufs=1))
        QT = pq.tile([P, NQCH, S], BF16, name="QT")
        KT = pq.tile([P, NKCH, S], BF16, name="KT")
        VN = pq.tile([P, NKC, HD], BF16, name="VN")
        OUTN = pq.tile([P, NQCH, S], BF16, name="OUTN")
        WO = pq.tile([P, NQCH, D], BF16, name="WO")
        ONESB = pq.tile([P, P], BF16, name="ONESB")
        ONESF = pq.tile([1, P], F32, name="ONESF")
        if causal:
            STAIR = pq.tile([P, 2 * SBLK], BF16, name="STAIR")

        # ---- phase A: projections + RoPE -----------------------------
        with tc.tile_pool(name="pw", bufs=1) as pw, \
             tc.tile_pool(name="pht", bufs=32) as pht, \
             tc.tile_pool(name="pcs", bufs=4) as pcs, \
             tc.tile_pool(name="pqt", bufs=8) as pqt, \
             tc.tile_pool(name="ptmp", bufs=6) as ptmp, \
             tc.tile_pool(name="ppqk", bufs=6, space="PSUM") as ppqk, \
             tc.tile_pool(name="ppv", bufs=2, space="PSUM") as ppv:
            WQ = pw.tile([P, DCH, DQ], BF16, name="WQ")
            WK = pw.tile([P, DCH, HD], BF16, name="WK")
            WV = pw.tile([P, DCH, HD], BF16, name="WV")

            for sb in range(NSBLK):
                ssl = slice(sb * SBLK, (sb + 1) * SBLK)
                hts = []
                for c in range(DCH):
                    ht = pht.tile([P, SBLK], BF16, name="ht")
                    hts.append(ht)
                    nc.sync.dma_start(out=ht, in_=hT[c * P:(c + 1) * P, ssl])
                    if sb == 0:
                        nc.sync.dma_start(out=WQ[:, c, :], in_=wq[c * P:(c + 1) * P, :])
                        nc.sync.dma_start(out=WK[:, c, :], in_=wk[c * P:(c + 1) * P, :])
                        nc.sync.dma_start(out=WV[:, c, :], in_=wv[c * P:(c + 1) * P, :])
                        if c == 0:
                            nc.sync.dma_start(out=ONESB, in_=onesb[:, :])
                            nc.sync.dma_start(out=ONESF, in_=onesf[:, :])
                            if causal:
                                nc.sync.dma_start(out=STAIR, in_=stair[:, :])
                    if sb == NSBLK - 1 and c < NQCH:
                        # o_proj weights are first needed deep into phase C;
                        # stream them late so they never delay hT/cos/sin
                        nc.sync.dma_start(out=WO[:, c, :],
                                          in_=wo[c * P:(c + 1) * P, :])
                COSb = pcs.tile([P, NKCH, SBLK], BF16, name="cosb")
                SINb = pcs.tile([P, NKCH, SBLK], BF16, name="sinb")
                for jj in range(NKCH):
                    nc.sync.dma_start(out=COSb[:, jj, :],
                                      in_=cosT[jj * P:(jj + 1) * P, ssl])
                    nc.sync.dma_start(out=SINb[:, jj, :],
                                      in_=sinT[jj * P:(jj + 1) * P, ssl])

                # Q/K accumulation, c-major (DMA-friendly: each ht used
                # 6x right after it lands)
                psq = [ppqk.tile([P, SBLK], F32, name="pp") for _ in range(NQCH)]
                psk = [ppqk.tile([P, SBLK], F32, name="pp") for _ in range(NKCH)]
                for c in range(DCH):
                    for i in range(NQCH):
                        nc.tensor.matmul(psq[i], lhsT=WQ[:, c, i * P:(i + 1) * P],
                                         rhs=hts[c], start=(c == 0),
                                         stop=(c == DCH - 1))
                    for j in range(NKCH):
                        nc.tensor.matmul(psk[j], lhsT=WK[:, c, j * P:(j + 1) * P],
                                         rhs=hts[c], start=(c == 0),
                                         stop=(c == DCH - 1))
                # fast ACT drains free the 6 banks; RoPE runs on DVE from
                # bf16 SBUF copies (2x mode) off the PE critical path
                qts = []
                for ps in psq + psk:
                    t = pqt.tile([P, SBLK], BF16, name="qt")
                    nc.scalar.copy(t, ps)
                    qts.append(t)

                def rope_pair(b0, b1, out0, out1):
                    c0 = COSb[:, 0, :]; c1 = COSb[:, 1, :]
                    s0 = SINb[:, 0, :]; s1 = SINb[:, 1, :]
                    t1 = ptmp.tile([P, SBLK], BF16, name="t")
                    t2 = ptmp.tile([P, SBLK], BF16, name="t")
                    nc.vector.tensor_mul(t1, b0, c0)
                    nc.vector.tensor_mul(t2, b1, s0)
                    nc.vector.tensor_sub(out0, t1, t2)
                    t3 = ptmp.tile([P, SBLK], BF16, name="t")
                    t4 = ptmp.tile([P, SBLK], BF16, name="t")
                    nc.vector.tensor_mul(t3, b1, c1)
                    nc.vector.tensor_mul(t4, b0, s1)
                    nc.vector.tensor_add(out1, t3, t4)

                for h in range(HEADS_PER_CORE):
                    rope_pair(qts[2 * h], qts[2 * h + 1],
                              QT[:, 2 * h, ssl], QT[:, 2 * h + 1, ssl])
                rope_pair(qts[NQCH], qts[NQCH + 1],
                          KT[:, 0, ssl], KT[:, 1, ssl])

                # direct V chains (PE work that covers the drains above)
                for si in range(SBLK // P):
                    # full-bank tile ([P,SBLK] f32 = 2KB) so two ppv bufs can
                    # never share a PSUM bank (PE-write + DVE-read collision)
                    psv = ppv.tile([P, SBLK], F32, name="pv")
                    for c in range(DCH):
                        nc.tensor.matmul(psv[:, :HD],
                                         lhsT=hts[c][:, si * P:(si + 1) * P],
                                         rhs=WV[:, c, :], start=(c == 0),
                                         stop=(c == DCH - 1))
                    nc.vector.tensor_copy(VN[:, sb * (SBLK // P) + si, :],
                                          psv[:, :HD])

        # ---- phase C+D: attention + norm + interleaved o_proj ---------
        with tc.tile_pool(name="pexp", bufs=5) as pexp, \
             tc.tile_pool(name="pou", bufs=4) as pou, \
             tc.tile_pool(name="pnrm", bufs=6) as pnrm, \
             tc.tile_pool(name="pfin", bufs=4) as pfin, \
             tc.tile_pool(name="pmask", bufs=4) as pmask, \
             tc.tile_pool(name="ps_s", bufs=3, space="PSUM") as ps_s, \
             tc.tile_pool(name="ps_o", bufs=2, space="PSUM") as ps_o, \
             tc.tile_pool(name="ps_d", bufs=1, space="PSUM") as ps_d, \
             tc.tile_pool(name="ps_f", bufs=2, space="PSUM") as ps_f:

            ONEC = ONESB[:, 0:1]
            ONERF = ONESF[0:1, :]

            def emit_norm(pend):
                ph, pqb, ou, den_sb = pend
                rd = pnrm.tile([1, SBLK], F32, name="rd")
                nc.vector.reciprocal_approx_fast(out=rd, in_=den_sb)
                psb = ps_f.tile([P, SBLK], F32, name="pf")
                nc.tensor.matmul(psb, lhsT=ONERF, rhs=rd, start=True, stop=True)
                rbc = pnrm.tile([P, SBLK], BF16, name="rbc")
                nc.vector.tensor_copy(rbc, psb)
                pqsl = slice(pqb * SBLK, (pqb + 1) * SBLK)
                for dvc in range(2):
                    nc.vector.tensor_mul(OUTN[:, 2 * ph + dvc, pqsl],
                                         ou[dvc], rbc)

            def emit_oproj_quarter(qb):
                for st in range(4 * qb, 4 * qb + 4):
                    stsl = slice(st * P, (st + 1) * P)
                    for nb in range(NSBLK):
                        psf = ps_f.tile([P, SBLK], F32, name="pf")
                        for dvc in range(NQCH):
                            nc.tensor.matmul(psf, lhsT=OUTN[:, dvc, stsl],
                                             rhs=WO[:, dvc, nb * SBLK:(nb + 1) * SBLK],
                                             start=(dvc == 0), stop=(dvc == NQCH - 1))
                        fsb = pfin.tile([P, SBLK], F32, name="fsb")
                        if (st + nb) % 2 == 0:
                            nc.vector.tensor_copy(fsb, psf)
                        else:
                            nc.scalar.copy(fsb, psf)
                        nc.sync.dma_start(out=outp[stsl, nb * SBLK:(nb + 1) * SBLK],
                                          in_=fsb)

            pending = None          # (h, qb, ou, den_sb) awaiting norm
            pending_oproj = None    # qb awaiting o_proj emission
            for h in range(HEADS_PER_CORE):
                for qb in range(NSBLK):
                    qsl = slice(qb * SBLK, (qb + 1) * SBLK)
                    klim = 4 * (qb + 1) if causal else NKC
                    pso = [ps_o.tile([P, SBLK], F32, name="po") for _ in range(2)]
                    pden = ps_d.tile([1, SBLK], F32, name="pd")
                    exs = [None] * klim

                    def emit_den_av(i):
                        ex = exs[i]
                        nc.tensor.matmul(pden, lhsT=ONEC, rhs=ex,
                                         start=(i == 0), stop=(i == klim - 1))
                        for dvc in range(2):
                            nc.tensor.matmul(pso[dvc],
                                             lhsT=VN[:, i, dvc * P:(dvc + 1) * P],
                                             rhs=ex, start=(i == 0),
                                             stop=(i == klim - 1))

                    for i in range(klim):
                        pss = ps_s.tile([P, SBLK], F32, name="ps")
                        for c in range(NKCH):
                            nc.tensor.matmul(pss,
                                             lhsT=KT[:, c, i * P:(i + 1) * P],
                                             rhs=QT[:, 2 * h + c, qsl],
                                             start=(c == 0), stop=(c == NKCH - 1))
                        if i == 1 and pending is not None:
                            emit_norm(pending)
                            pending = None
                        ex = pexp.tile([P, SBLK], BF16, name="ex")
                        nc.scalar.activation(ex, pss, EXP, scale=1.0 / 16.0)
                        if causal and i >= 4 * qb:
                            delta = 128 * i - 512 * qb
                            nc.vector.tensor_mul(ex, ex,
                                                 STAIR[:, 512 - delta:1024 - delta])
                        if not causal:
                            mt = pmask.tile([P, SBLK], BF16, name="mt")
                            nc.sync.dma_start(out=mt,
                                              in_=maskT[i * P:(i + 1) * P, qsl])
                            nc.vector.tensor_mul(ex, ex, mt)
                        exs[i] = ex
                        if i == 3 and pending_oproj is not None:
                            emit_oproj_quarter(pending_oproj)
                            pending_oproj = None
                        if i >= 2:
                            emit_den_av(i - 2)
                    emit_den_av(klim - 2)
                    emit_den_av(klim - 1)

                    # immediate drains: free pso/pden quickly
                    ou = [pou.tile([P, SBLK], BF16, name="ou") for _ in range(2)]
                    for dvc in range(2):
                        nc.vector.tensor_copy(ou[dvc], pso[dvc])
                    den_sb = pnrm.tile([1, SBLK], F32, name="dsb")
                    nc.scalar.copy(den_sb, pden)
                    pending = (h, qb, ou, den_sb)
                    if h == 1:
                        pending_oproj = qb
            emit_norm(pending)
            emit_oproj_quarter(pending_oproj)

    nc.finalize()
    return nc


def _get_nc(causal: bool):
    key = bool(causal)
    if key not in _BUILD_CACHE:
        _BUILD_CACHE[key] = _build(causal)
    return _BUILD_CACHE[key]


def _rope_tables(position_ids_b):
    # cosT/sinT: [HD, S] bf16, transposed layout for the [d, s] dataflow
    pos = np.asarray(position_ids_b, dtype=np.float64)
    inv = 1.0 / (ROPE_BASE ** (np.arange(0, HD, 2, dtype=np.float64) / HD))
    f = pos[:, None] * inv[None, :]            # [S, HD/2]
    emb = np.concatenate([f, f], axis=1)       # [S, HD]
    cosT = np.ascontiguousarray(np.cos(emb).T).astype(ml_dtypes.bfloat16)
    sinT = np.ascontiguousarray(np.sin(emb).T).astype(ml_dtypes.bfloat16)
    return cosT, sinT


def _is_causal(attention_mask):
    m = np.asarray(attention_mask)
    if m.shape != (B, 1, S, S):
        return False
    tri = np.tril(np.ones((S, S), dtype=bool))
    canon = np.where(tri, np.float32(0.0), np.float32(-1e9))
    return all(np.array_equal(m[b, 0], canon) for b in range(B))


_ONESB_NP = np.ones((P, P), dtype=ml_dtypes.bfloat16)
_ONESF_NP = np.ones((1, P), dtype=np.float32)


def _stair():
    # multiplicative staircase: stair[p, j] = 1 if (j - 512) >= p else 0
    j = np.arange(2 * SBLK)[None, :] - SBLK
    p = np.arange(P)[:, None]
    return np.where(j >= p, 1.0, 0.0).astype(ml_dtypes.bfloat16)


def kernel(hidden_state, attention_mask, position_ids, Wq, Wk, Wv, Wo,
           _trace=False, _tmpdir=None):
    global LAST_EXEC_TIME_NS
    hidden_state = np.asarray(hidden_state, dtype=np.float32)
    Wq = np.asarray(Wq, dtype=np.float32)
    Wk = np.asarray(Wk, dtype=np.float32)
    Wv = np.asarray(Wv, dtype=np.float32)
    Wo = np.asarray(Wo, dtype=np.float32)

    causal = _is_causal(attention_mask)
    nc = _get_nc(causal)

    stair = _stair() if causal else None
    wk_b = Wk.astype(ml_dtypes.bfloat16)
    wv_b = Wv.astype(ml_dtypes.bfloat16)
    per_batch = {}
    for b in range(B):
        hTb = np.ascontiguousarray(hidden_state[b].T).astype(ml_dtypes.bfloat16)
        cosTb, sinTb = _rope_tables(position_ids[b])
        mb = None
        if not causal:
            mb = np.ascontiguousarray(
                np.exp(np.asarray(attention_mask, dtype=np.float64)[b, 0].T)
            ).astype(ml_dtypes.bfloat16)
        per_batch[b] = (hTb, cosTb, sinTb, mb)

    in_maps = []
    for core in range(8):
        b = core // 4
        hp = core % 4
        hTb, cosTb, sinTb, mb = per_batch[b]
        im = {
            "hT": hTb,
            "onesb": _ONESB_NP,
            "onesf": _ONESF_NP,
            "wq": np.ascontiguousarray(
                Wq[:, hp * DQ:(hp + 1) * DQ]).astype(ml_dtypes.bfloat16),
            "wk": wk_b,
            "wv": wv_b,
            "wo": np.ascontiguousarray(
                Wo[hp * DQ:(hp + 1) * DQ, :]).astype(ml_dtypes.bfloat16),
            "cosT": cosTb,
            "sinT": sinTb,
        }
        if causal:
            im["stair"] = stair
        else:
            im["emaskT"] = mb
        in_maps.append(im)

    res = run_bass_kernel_spmd(nc, in_maps, core_ids=list(range(8)),
                               trace=_trace, tmpdir=_tmpdir)
    LAST_EXEC_TIME_NS = res.exec_time_ns

    out = np.empty((B, S, D), dtype=np.float32)
    for b in range(B):
        acc = res.results[4 * b]["out_partial"].astype(np.float32).copy()
        for hp in range(1, 4):
            acc += res.results[4 * b + hp]["out_partial"]
        out[b] = acc
    return out


# revision 11
# speedup vs baseline: 1.4925x; 1.0397x over previous
"""GemmaAttention (B=2, S=2048, D=2048, H=8, KV=1, HD=256) on 8 trn2 NeuronCores.

Sharding: DP=2 over batch x TP=4 over head-pairs. Core c handles batch c//4 and
heads {2*(c%4), 2*(c%4)+1}. Each core computes its partial o_proj output
(row-parallel Wo); the host sums the 4 partials per batch (the all-reduce is
folded into the host-side unshard).

All matmuls run in bf16 (fp32 PSUM accumulate): fp32r streams at ~half the
bf16 column rate on the PE (389ns vs 213ns per N=512 matmul), so bf16 nearly
doubles tensor-engine throughput. rel-err budget is 2e-2; bf16 lands ~3e-3.

Dataflow per core:
  phase A (projections, per 512-col s-block):
    QT[dq,s], KT[dk,s] accumulate c-major over 16 D-chunks (6 PSUM banks),
    drained via ACT copy (psum->bf16 SBUF, frees banks fast) then RoPE on DVE
    in bf16 (2x mode). V[s,dv] computed directly (lhsT = hT chunk) in 128-row
    chains on 2 more banks -- this PE work covers the RoPE drain latency so
    the PE never idles at s-block boundaries.
  phase C (attention, per (head, q-block) item, software-pipelined):
    scoresT[k,q] = KT_chunk.T @ QT with a skew-2 pipeline: S(i) issues two
    iterations ahead of DEN(i)/AV(i) so the ACT exp (+DVE causal staircase)
    latency is hidden behind PE work. Denominators accumulate on the PE
    (ONEC.T @ ex into a [1,512] PSUM row). Per-item normalization is deferred
    one item (reciprocal_approx_fast + ONES-broadcast matmul + DVE scale) and
    o_proj quarters are injected into the next item's PE stream.
"""

import numpy as np
import ml_dtypes

import concourse.bass as bass
import concourse.tile as tile
import concourse.mybir as mybir
from concourse import bacc
from concourse.bass_utils import run_bass_kernel_spmd

P = 128
B, S, D = 2, 2048, 2048
H, KV, HD = 8, 1, 256
ROPE_BASE = 10000.0

HEADS_PER_CORE = 2
DQ = HEADS_PER_CORE * HD          # 512 q-dims per core
DCH = D // P                      # 16 contraction chunks
SBLK = 512                        # s-tile for projection rhs / q-tile
NSBLK = S // SBLK                 # 4
NKC = S // P                      # 16 key chunks
NQCH = DQ // P                    # 4 QT partition chunks
NKCH = HD // P                    # 2 KT partition chunks

F32 = mybir.dt.float32
BF16 = mybir.dt.bfloat16
EXP = mybir.ActivationFunctionType.Exp

LAST_EXEC_TIME_NS = None

_BUILD_CACHE = {}


def _build(causal: bool):
    nc = bacc.Bacc()

    hT = nc.declare_dram_parameter("hT", [D, S], BF16, isOutput=False)
    wq = nc.declare_dram_parameter("wq", [D, DQ], BF16, isOutput=False)
    wk = nc.declare_dram_parameter("wk", [D, HD], BF16, isOutput=False)
    wv = nc.declare_dram_parameter("wv", [D, HD], BF16, isOutput=False)
    wo = nc.declare_dram_parameter("wo", [DQ, D], BF16, isOutput=False)
    cosT = nc.declare_dram_parameter("cosT", [HD, S], BF16, isOutput=False)
    sinT = nc.declare_dram_parameter("sinT", [HD, S], BF16, isOutput=False)
    onesb = nc.declare_dram_parameter("onesb", [P, P], BF16, isOutput=False)
    onesf = nc.declare_dram_parameter("onesf", [1, P], F32, isOutput=False)
    if causal:
        stair = nc.declare_dram_parameter("stair", [P, 2 * SBLK], BF16,
                                          isOutput=False)
    else:
        maskT = nc.declare_dram_parameter("emaskT", [S, S], BF16, isOutput=False)
    outp = nc.declare_dram_parameter("out_partial", [S, D], F32, isOutput=True)

    from contextlib import ExitStack
    with tile.TileContext(nc) as tc, ExitStack() as ctx:
        pq = ctx.enter_context(tc.tile_pool(name="pq", bufs=1))
        QT = pq.tile([P, NQCH, S], BF16, name="QT")
        KT = pq.tile([P, NKCH, S], BF16, name="KT")
        VN = pq.tile([P, NKC, HD], BF16, name="VN")
        OUTN = pq.tile([P, NQCH, S], BF16, name="OUTN")
        WO = pq.tile([P, NQCH, D], BF16, name="WO")
        ONESB = pq.tile([P, P], BF16, name="ONESB")
        ONESF = pq.tile([1, P], F32, name="ONESF")
        if causal:
            STAIR = pq.tile([P, 2 * SBLK], BF16, name="STAIR")

        # ---- phase A: projections + RoPE -----------------------------
        with tc.tile_pool(name="pw", bufs=1) as pw, \
             tc.tile_pool(name="pht", bufs=32) as pht, \
             tc.tile_pool(name="pcs", bufs=4) as pcs, \
             tc.tile_pool(name="pqt", bufs=8) as pqt, \
             tc.tile_pool(name="ptmp", bufs=6) as ptmp, \
             tc.tile_pool(name="ppqk", bufs=6, space="PSUM") as ppqk, \
             tc.tile_pool(name="ppv", bufs=2, space="PSUM") as ppv:
            WQ = pw.tile([P, DCH, DQ], BF16, name="WQ")
            WK = pw.tile([P, DCH, HD], BF16, name="WK")
            WV = pw.tile([P, DCH, HD], BF16, name="WV")

            for sb in range(NSBLK):
                ssl = slice(sb * SBLK, (sb + 1) * SBLK)
                # spread input DMAs over several engine queues so the early
                # projection chunks are never paced by a single DMA ring
                hts = []
                for c in range(DCH):
                    ht = pht.tile([P, SBLK], BF16, name="ht")
                    hts.append(ht)
                    heng = nc.sync if c % 2 == 0 else nc.gpsimd
                    heng.dma_start(out=ht, in_=hT[c * P:(c + 1) * P, ssl])
                    if sb == 0:
                        nc.scalar.dma_start(out=WQ[:, c, :], in_=wq[c * P:(c + 1) * P, :])
                        nc.scalar.dma_start(out=WK[:, c, :], in_=wk[c * P:(c + 1) * P, :])
                        nc.scalar.dma_start(out=WV[:, c, :], in_=wv[c * P:(c + 1) * P, :])
                        if c == 0:
                            nc.gpsimd.dma_start(out=ONESB, in_=onesb[:, :])
                            nc.gpsimd.dma_start(out=ONESF, in_=onesf[:, :])
                            if causal:
                                nc.gpsimd.dma_start(out=STAIR, in_=stair[:, :])
                    if sb == NSBLK - 1 and c < NQCH:
                        # o_proj weights are first needed deep into phase C;
                        # stream them late so they never delay hT/cos/sin
                        nc.sync.dma_start(out=WO[:, c, :],
                                          in_=wo[c * P:(c + 1) * P, :])
                COSb = pcs.tile([P, NKCH, SBLK], BF16, name="cosb")
                SINb = pcs.tile([P, NKCH, SBLK], BF16, name="sinb")
                for jj in range(NKCH):
                    nc.gpsimd.dma_start(out=COSb[:, jj, :],
                                         in_=cosT[jj * P:(jj + 1) * P, ssl])
                    nc.gpsimd.dma_start(out=SINb[:, jj, :],
                                         in_=sinT[jj * P:(jj + 1) * P, ssl])

                # Q/K accumulation, c-major (DMA-friendly: each ht used
                # 6x right after it lands)
                psq = [ppqk.tile([P, SBLK], F32, name="pp") for _ in range(NQCH)]
                psk = [ppqk.tile([P, SBLK], F32, name="pp") for _ in range(NKCH)]
                for c in range(DCH):
                    for i in range(NQCH):
                        nc.tensor.matmul(psq[i], lhsT=WQ[:, c, i * P:(i + 1) * P],
                                         rhs=hts[c], start=(c == 0),
                                         stop=(c == DCH - 1))
                    for j in range(NKCH):
                        nc.tensor.matmul(psk[j], lhsT=WK[:, c, j * P:(j + 1) * P],
                                         rhs=hts[c], start=(c == 0),
                                         stop=(c == DCH - 1))
                # fast ACT drains free the 6 banks; RoPE runs on DVE from
                # bf16 SBUF copies (2x mode) off the PE critical path
                qts = []
                for ps in psq + psk:
                    t = pqt.tile([P, SBLK], BF16, name="qt")
                    nc.scalar.copy(t, ps)
                    qts.append(t)

                def rope_pair(b0, b1, out0, out1):
                    c0 = COSb[:, 0, :]; c1 = COSb[:, 1, :]
                    s0 = SINb[:, 0, :]; s1 = SINb[:, 1, :]
                    t1 = ptmp.tile([P, SBLK], BF16, name="t")
                    t2 = ptmp.tile([P, SBLK], BF16, name="t")
                    nc.vector.tensor_mul(t1, b0, c0)
                    nc.vector.tensor_mul(t2, b1, s0)
                    nc.vector.tensor_sub(out0, t1, t2)
                    t3 = ptmp.tile([P, SBLK], BF16, name="t")
                    t4 = ptmp.tile([P, SBLK], BF16, name="t")
                    nc.vector.tensor_mul(t3, b1, c1)
                    nc.vector.tensor_mul(t4, b0, s1)
                    nc.vector.tensor_add(out1, t3, t4)

                for h in range(HEADS_PER_CORE):
                    rope_pair(qts[2 * h], qts[2 * h + 1],
                              QT[:, 2 * h, ssl], QT[:, 2 * h + 1, ssl])
                rope_pair(qts[NQCH], qts[NQCH + 1],
                          KT[:, 0, ssl], KT[:, 1, ssl])

                # direct V chains (PE work that covers the drains above)
                for si in range(SBLK // P):
                    # full-bank tile ([P,SBLK] f32 = 2KB) so two ppv bufs can
                    # never share a PSUM bank (PE-write + DVE-read collision)
                    psv = ppv.tile([P, SBLK], F32, name="pv")
                    for c in range(DCH):
                        nc.tensor.matmul(psv[:, :HD],
                                         lhsT=hts[c][:, si * P:(si + 1) * P],
                                         rhs=WV[:, c, :], start=(c == 0),
                                         stop=(c == DCH - 1))
                    nc.vector.tensor_copy(VN[:, sb * (SBLK // P) + si, :],
                                          psv[:, :HD])

        # ---- phase C+D: attention + norm + interleaved o_proj ---------
        with tc.tile_pool(name="pexp", bufs=17) as pexp, \
             tc.tile_pool(name="pou", bufs=4) as pou, \
             tc.tile_pool(name="pnrm", bufs=6) as pnrm, \
             tc.tile_pool(name="pfin", bufs=4) as pfin, \
             tc.tile_pool(name="pmask", bufs=4) as pmask, \
             tc.tile_pool(name="ps_s", bufs=3, space="PSUM") as ps_s, \
             tc.tile_pool(name="ps_o", bufs=2, space="PSUM") as ps_o, \
             tc.tile_pool(name="ps_d", bufs=1, space="PSUM") as ps_d, \
             tc.tile_pool(name="ps_f", bufs=2, space="PSUM") as ps_f:

            ONEC = ONESB[:, 0:1]
            ONERF = ONESF[0:1, :]

            def emit_norm(pend):
                ph, pqb, ou, den_sb = pend
                rd = pnrm.tile([1, SBLK], F32, name="rd")
                nc.vector.reciprocal_approx_fast(out=rd, in_=den_sb)
                rdb = pnrm.tile([1, SBLK], BF16, name="rdb")
                nc.vector.tensor_copy(rdb, rd)
                psb = ps_f.tile([P, SBLK], F32, name="pf")
                nc.tensor.matmul(psb, lhsT=ONESB[0:1, :], rhs=rdb,
                                 start=True, stop=True)
                rbc = pnrm.tile([P, SBLK], BF16, name="rbc")
                nc.vector.tensor_copy(rbc, psb)
                pqsl = slice(pqb * SBLK, (pqb + 1) * SBLK)
                for dvc in range(2):
                    nc.vector.tensor_mul(OUTN[:, 2 * ph + dvc, pqsl],
                                         ou[dvc], rbc)

            def emit_oproj_quarter(qb):
                for st in range(4 * qb, 4 * qb + 4):
                    stsl = slice(st * P, (st + 1) * P)
                    for nb in range(NSBLK):
                        psf = ps_f.tile([P, SBLK], F32, name="pf")
                        for dvc in range(NQCH):
                            nc.tensor.matmul(psf, lhsT=OUTN[:, dvc, stsl],
                                             rhs=WO[:, dvc, nb * SBLK:(nb + 1) * SBLK],
                                             start=(dvc == 0), stop=(dvc == NQCH - 1))
                        fsb = pfin.tile([P, SBLK], F32, name="fsb")
                        if (st + nb) % 2 == 0:
                            nc.vector.tensor_copy(fsb, psf)
                        else:
                            nc.scalar.copy(fsb, psf)
                        nc.sync.dma_start(out=outp[stsl, nb * SBLK:(nb + 1) * SBLK],
                                          in_=fsb)

            pending = None          # (h, qb, ou, den_sb) awaiting norm
            pending_oproj = None    # qb awaiting o_proj emission
            for h in range(HEADS_PER_CORE):
                for qb in range(NSBLK):
                    qsl = slice(qb * SBLK, (qb + 1) * SBLK)
                    klim = 4 * (qb + 1) if causal else NKC
                    pso = [ps_o.tile([P, SBLK], F32, name="po") for _ in range(2)]
                    pden = ps_d.tile([1, SBLK], F32, name="pd")
                    exs = [None] * klim

                    def emit_av(i):
                        ex = exs[i]
                        for dvc in range(2):
                            nc.tensor.matmul(pso[dvc],
                                             lhsT=VN[:, i, dvc * P:(dvc + 1) * P],
                                             rhs=ex, start=(i == 0),
                                             stop=(i == klim - 1))

                    for i in range(klim):
                        pss = ps_s.tile([P, SBLK], F32, name="ps")
                        for c in range(NKCH):
                            nc.tensor.matmul(pss,
                                             lhsT=KT[:, c, i * P:(i + 1) * P],
                                             rhs=QT[:, 2 * h + c, qsl],
                                             start=(c == 0), stop=(c == NKCH - 1))
                        if i == 1 and pending is not None:
                            emit_norm(pending)
                            pending = None
                        ex = pexp.tile([P, SBLK], BF16, name="ex")
                        nc.scalar.activation(ex, pss, EXP, scale=1.0 / 16.0)
                        if causal and i >= 4 * qb:
                            delta = 128 * i - 512 * qb
                            nc.vector.tensor_mul(ex, ex,
                                                 STAIR[:, 512 - delta:1024 - delta])
                        if not causal:
                            mt = pmask.tile([P, SBLK], BF16, name="mt")
                            nc.sync.dma_start(out=mt,
                                              in_=maskT[i * P:(i + 1) * P, qsl])
                            nc.vector.tensor_mul(ex, ex, mt)
                        exs[i] = ex
                        if i == 3 and pending_oproj is not None:
                            emit_oproj_quarter(pending_oproj)
                            pending_oproj = None
                        if i >= 2:
                            emit_av(i - 2)
                    emit_av(klim - 2)
                    emit_av(klim - 1)
                    # batched denominator reduce: ONEC stays stationary, so
                    # these klim matmuls stream back-to-back with no
                    # LDWEIGHTS churn and no unsatisfied waits
                    for i in range(klim):
                        nc.tensor.matmul(pden, lhsT=ONEC, rhs=exs[i],
                                         start=(i == 0), stop=(i == klim - 1))

                    # immediate drains: free pso/pden quickly
                    ou = [pou.tile([P, SBLK], BF16, name="ou") for _ in range(2)]
                    for dvc in range(2):
                        nc.vector.tensor_copy(ou[dvc], pso[dvc])
                    den_sb = pnrm.tile([1, SBLK], F32, name="dsb")
                    nc.scalar.copy(den_sb, pden)
                    pending = (h, qb, ou, den_sb)
                    if h == 1:
                        pending_oproj = qb
            emit_norm(pending)
            emit_oproj_quarter(pending_oproj)

    nc.finalize()
    return nc


def _get_nc(causal: bool):
    key = bool(causal)
    if key not in _BUILD_CACHE:
        _BUILD_CACHE[key] = _build(causal)
    return _BUILD_CACHE[key]


def _rope_tables(position_ids_b):
    # cosT/sinT: [HD, S] bf16, transposed layout for the [d, s] dataflow
    pos = np.asarray(position_ids_b, dtype=np.float64)
    inv = 1.0 / (ROPE_BASE ** (np.arange(0, HD, 2, dtype=np.float64) / HD))
    f = pos[:, None] * inv[None, :]            # [S, HD/2]
    emb = np.concatenate([f, f], axis=1)       # [S, HD]
    cosT = np.ascontiguousarray(np.cos(emb).T).astype(ml_dtypes.bfloat16)
    sinT = np.ascontiguousarray(np.sin(emb).T).astype(ml_dtypes.bfloat16)
    return cosT, sinT


def _is_causal(attention_mask):
    m = np.asarray(attention_mask)
    if m.shape != (B, 1, S, S):
        return False
    tri = np.tril(np.ones((S, S), dtype=bool))
    canon = np.where(tri, np.float32(0.0), np.float32(-1e9))
    return all(np.array_equal(m[b, 0], canon) for b in range(B))


_ONESB_NP = np.ones((P, P), dtype=ml_dtypes.bfloat16)
_ONESF_NP = np.ones((1, P), dtype=np.float32)


def _stair():
    # multiplicative staircase: stair[p, j] = 1 if (j - 512) >= p else 0
    j = np.arange(2 * SBLK)[None, :] - SBLK
    p = np.arange(P)[:, None]
    return np.where(j >= p, 1.0, 0.0).astype(ml_dtypes.bfloat16)


def kernel(hidden_state, attention_mask, position_ids, Wq, Wk, Wv, Wo,
           _trace=False, _tmpdir=None):
    global LAST_EXEC_TIME_NS
    hidden_state = np.asarray(hidden_state, dtype=np.float32)
    Wq = np.asarray(Wq, dtype=np.float32)
    Wk = np.asarray(Wk, dtype=np.float32)
    Wv = np.asarray(Wv, dtype=np.float32)
    Wo = np.asarray(Wo, dtype=np.float32)

    causal = _is_causal(attention_mask)
    nc = _get_nc(causal)

    stair = _stair() if causal else None
    wk_b = Wk.astype(ml_dtypes.bfloat16)
    wv_b = Wv.astype(ml_dtypes.bfloat16)
    per_batch = {}
    for b in range(B):
        hTb = np.ascontiguousarray(hidden_state[b].T).astype(ml_dtypes.bfloat16)
        cosTb, sinTb = _rope_tables(position_ids[b])
        mb = None
        if not causal:
            mb = np.ascontiguousarray(
                np.exp(np.asarray(attention_mask, dtype=np.float64)[b, 0].T)
            ).astype(ml_dtypes.bfloat16)
        per_batch[b] = (hTb, cosTb, sinTb, mb)

    in_maps = []
    for core in range(8):
        b = core // 4
        hp = core % 4
        hTb, cosTb, sinTb, mb = per_batch[b]
        im = {
            "hT": hTb,
            "onesb": _ONESB_NP,
            "onesf": _ONESF_NP,
            "wq": np.ascontiguousarray(
                Wq[:, hp * DQ:(hp + 1) * DQ]).astype(ml_dtypes.bfloat16),
            "wk": wk_b,
            "wv": wv_b,
            "wo": np.ascontiguousarray(
                Wo[hp * DQ:(hp + 1) * DQ, :]).astype(ml_dtypes.bfloat16),
            "cosT": cosTb,
            "sinT": sinTb,
        }
        if causal:
            im["stair"] = stair
        else:
            im["emaskT"] = mb
        in_maps.append(im)

    res = run_bass_kernel_spmd(nc, in_maps, core_ids=list(range(8)),
                               trace=_trace, tmpdir=_tmpdir)
    LAST_EXEC_TIME_NS = res.exec_time_ns

    out = np.empty((B, S, D), dtype=np.float32)
    for b in range(B):
        acc = res.results[4 * b]["out_partial"].astype(np.float32).copy()
        for hp in range(1, 4):
            acc += res.results[4 * b + hp]["out_partial"]
        out[b] = acc
    return out


# revision 14
# speedup vs baseline: 1.5635x; 1.0476x over previous
"""GemmaAttention (B=2, S=2048, D=2048, H=8, KV=1, HD=256) on 8 trn2 NeuronCores.

Sharding: DP=2 over batch x TP=4 over head-pairs. Core c handles batch c//4 and
heads {2*(c%4), 2*(c%4)+1}. Each core computes its partial o_proj output
(row-parallel Wo); the host sums the 4 partials per batch (the all-reduce is
folded into the host-side unshard).

All matmuls run in bf16 (fp32 PSUM accumulate): fp32r streams at ~half the
bf16 column rate on the PE (389ns vs 213ns per N=512 matmul), so bf16 nearly
doubles tensor-engine throughput. rel-err budget is 2e-2; bf16 lands ~3e-3.

Dataflow per core:
  phase A (projections, per 512-col s-block):
    QT[dq,s], KT[dk,s] accumulate c-major over 16 D-chunks (6 PSUM banks),
    drained via ACT copy (psum->bf16 SBUF, frees banks fast) then RoPE on DVE
    in bf16 (2x mode). V[s,dv] computed directly (lhsT = hT chunk) in 128-row
    chains on 2 more banks -- this PE work covers the RoPE drain latency so
    the PE never idles at s-block boundaries.
  phase C (attention, per (head, q-block) item, software-pipelined):
    scoresT[k,q] = KT_chunk.T @ QT with a skew-2 pipeline: S(i) issues two
    iterations ahead of DEN(i)/AV(i) so the ACT exp (+DVE causal staircase)
    latency is hidden behind PE work. Denominators accumulate on the PE
    (ONEC.T @ ex into a [1,512] PSUM row). Per-item normalization is deferred
    one item (reciprocal_approx_fast + ONES-broadcast matmul + DVE scale) and
    o_proj quarters are injected into the next item's PE stream.
"""

import numpy as np
import ml_dtypes

import concourse.bass as bass
import concourse.tile as tile
import concourse.mybir as mybir
from concourse import bacc
from concourse.bass_utils import run_bass_kernel_spmd

P = 128
B, S, D = 2, 2048, 2048
H, KV, HD = 8, 1, 256
ROPE_BASE = 10000.0

HEADS_PER_CORE = 2
DQ = HEADS_PER_CORE * HD          # 512 q-dims per core
DCH = D // P                      # 16 contraction chunks
SBLK = 512                        # s-tile for projection rhs / q-tile
NSBLK = S // SBLK                 # 4
NKC = S // P                      # 16 key chunks
NQCH = DQ // P                    # 4 QT partition chunks
NKCH = HD // P                    # 2 KT partition chunks

F32 = mybir.dt.float32
BF16 = mybir.dt.bfloat16
EXP = mybir.ActivationFunctionType.Exp

LAST_EXEC_TIME_NS = None

_BUILD_CACHE = {}


def _build(causal: bool):
    nc = bacc.Bacc()

    hT = nc.declare_dram_parameter("hT", [D, S], BF16, isOutput=False)
    wq = nc.declare_dram_parameter("wq", [D, DQ], BF16, isOutput=False)
    wk = nc.declare_dram_parameter("wk", [D, HD], BF16, isOutput=False)
    wv = nc.declare_dram_parameter("wv", [D, HD], BF16, isOutput=False)
    wo = nc.declare_dram_parameter("wo", [DQ, D], BF16, isOutput=False)
    cosT = nc.declare_dram_parameter("cosT", [HD, S], BF16, isOutput=False)
    sinT = nc.declare_dram_parameter("sinT", [HD, S], BF16, isOutput=False)
    onesb = nc.declare_dram_parameter("onesb", [P, P], BF16, isOutput=False)
    onesf = nc.declare_dram_parameter("onesf", [1, P], F32, isOutput=False)
    if causal:
        stair = nc.declare_dram_parameter("stair", [P, 2 * SBLK], BF16,
                                          isOutput=False)
    else:
        maskT = nc.declare_dram_parameter("emaskT", [S, S], BF16, isOutput=False)
    outp = nc.declare_dram_parameter("out_partial", [S, D], F32, isOutput=True)

    from contextlib import ExitStack
    with tile.TileContext(nc) as tc, ExitStack() as ctx:
        pq = ctx.enter_context(tc.tile_pool(name="pq", bufs=1))
        QT = pq.tile([P, NQCH, S], BF16, name="QT")
        KT = pq.tile([P, NKCH, S], BF16, name="KT")
        VN = pq.tile([P, NKC, HD], BF16, name="VN")
        OUTN = pq.tile([P, NQCH, S], BF16, name="OUTN")
        WO = pq.tile([P, NQCH, D], BF16, name="WO")
        ONESB = pq.tile([P, P], BF16, name="ONESB")
        ONESF = pq.tile([1, P], F32, name="ONESF")
        if causal:
            STAIR = pq.tile([P, 2 * SBLK], BF16, name="STAIR")

        # ---- phase A: projections + RoPE -----------------------------
        with tc.tile_pool(name="pw", bufs=1) as pw, \
             tc.tile_pool(name="pht", bufs=32) as pht, \
             tc.tile_pool(name="pcs", bufs=4) as pcs, \
             tc.tile_pool(name="pqt", bufs=8) as pqt, \
             tc.tile_pool(name="ptmp", bufs=6) as ptmp, \
             tc.tile_pool(name="ppqk", bufs=6, space="PSUM") as ppqk, \
             tc.tile_pool(name="ppv", bufs=2, space="PSUM") as ppv:
            WQ = pw.tile([P, DCH, DQ], BF16, name="WQ")
            WK = pw.tile([P, DCH, HD], BF16, name="WK")
            WV = pw.tile([P, DCH, HD], BF16, name="WV")

            for sb in range(NSBLK):
                ssl = slice(sb * SBLK, (sb + 1) * SBLK)
                # spread input DMAs over several engine queues so the early
                # projection chunks are never paced by a single DMA ring
                hts = []
                for c in range(DCH):
                    ht = pht.tile([P, SBLK], BF16, name="ht")
                    hts.append(ht)
                    nc.sync.dma_start(out=ht, in_=hT[c * P:(c + 1) * P, ssl])
                    if sb == 0:
                        nc.sync.dma_start(out=WQ[:, c, :], in_=wq[c * P:(c + 1) * P, :])
                        nc.sync.dma_start(out=WK[:, c, :], in_=wk[c * P:(c + 1) * P, :])
                        nc.sync.dma_start(out=WV[:, c, :], in_=wv[c * P:(c + 1) * P, :])
                        if c == 0:
                            nc.sync.dma_start(out=ONESB, in_=onesb[:, :])
                            nc.sync.dma_start(out=ONESF, in_=onesf[:, :])
                            if causal:
                                nc.sync.dma_start(out=STAIR, in_=stair[:, :])
                    if sb == NSBLK - 1 and c < NQCH:
                        # o_proj weights are first needed deep into phase C;
                        # stream them late so they never delay hT/cos/sin
                        nc.sync.dma_start(out=WO[:, c, :],
                                          in_=wo[c * P:(c + 1) * P, :])
                COSb = pcs.tile([P, NKCH, SBLK], BF16, name="cosb")
                SINb = pcs.tile([P, NKCH, SBLK], BF16, name="sinb")
                for jj in range(NKCH):
                    nc.sync.dma_start(out=COSb[:, jj, :],
                                      in_=cosT[jj * P:(jj + 1) * P, ssl])
                    nc.sync.dma_start(out=SINb[:, jj, :],
                                      in_=sinT[jj * P:(jj + 1) * P, ssl])

                # Q/K accumulation, c-major (DMA-friendly: each ht used
                # 6x right after it lands)
                psq = [ppqk.tile([P, SBLK], F32, name="pp") for _ in range(NQCH)]
                psk = [ppqk.tile([P, SBLK], F32, name="pp") for _ in range(NKCH)]
                for c in range(DCH):
                    for i in range(NQCH):
                        nc.tensor.matmul(psq[i], lhsT=WQ[:, c, i * P:(i + 1) * P],
                                         rhs=hts[c], start=(c == 0),
                                         stop=(c == DCH - 1))
                    for j in range(NKCH):
                        nc.tensor.matmul(psk[j], lhsT=WK[:, c, j * P:(j + 1) * P],
                                         rhs=hts[c], start=(c == 0),
                                         stop=(c == DCH - 1))
                # fast ACT drains free the 6 banks; RoPE runs on DVE from
                # bf16 SBUF copies (2x mode) off the PE critical path
                qts = []
                for ps in psq + psk:
                    t = pqt.tile([P, SBLK], BF16, name="qt")
                    nc.scalar.copy(t, ps)
                    qts.append(t)

                def rope_pair(b0, b1, out0, out1):
                    c0 = COSb[:, 0, :]; c1 = COSb[:, 1, :]
                    s0 = SINb[:, 0, :]; s1 = SINb[:, 1, :]
                    t1 = ptmp.tile([P, SBLK], BF16, name="t")
                    t2 = ptmp.tile([P, SBLK], BF16, name="t")
                    nc.vector.tensor_mul(t1, b0, c0)
                    nc.vector.tensor_mul(t2, b1, s0)
                    nc.vector.tensor_sub(out0, t1, t2)
                    t3 = ptmp.tile([P, SBLK], BF16, name="t")
                    t4 = ptmp.tile([P, SBLK], BF16, name="t")
                    nc.vector.tensor_mul(t3, b1, c1)
                    nc.vector.tensor_mul(t4, b0, s1)
                    nc.vector.tensor_add(out1, t3, t4)

                for h in range(HEADS_PER_CORE):
                    rope_pair(qts[2 * h], qts[2 * h + 1],
                              QT[:, 2 * h, ssl], QT[:, 2 * h + 1, ssl])
                rope_pair(qts[NQCH], qts[NQCH + 1],
                          KT[:, 0, ssl], KT[:, 1, ssl])

                # direct V chains (PE work that covers the drains above)
                for si in range(SBLK // P):
                    # full-bank tile ([P,SBLK] f32 = 2KB) so two ppv bufs can
                    # never share a PSUM bank (PE-write + DVE-read collision)
                    psv = ppv.tile([P, SBLK], F32, name="pv")
                    for c in range(DCH):
                        nc.tensor.matmul(psv[:, :HD],
                                         lhsT=hts[c][:, si * P:(si + 1) * P],
                                         rhs=WV[:, c, :], start=(c == 0),
                                         stop=(c == DCH - 1))
                    nc.vector.tensor_copy(VN[:, sb * (SBLK // P) + si, :],
                                          psv[:, :HD])

        # ---- phase C+D: attention + norm + interleaved o_proj ---------
        with tc.tile_pool(name="pexp", bufs=17) as pexp, \
             tc.tile_pool(name="pou", bufs=4) as pou, \
             tc.tile_pool(name="pnrm", bufs=6) as pnrm, \
             tc.tile_pool(name="pfin", bufs=4) as pfin, \
             tc.tile_pool(name="pmask", bufs=4) as pmask, \
             tc.tile_pool(name="ps_s", bufs=3, space="PSUM") as ps_s, \
             tc.tile_pool(name="ps_o", bufs=2, space="PSUM") as ps_o, \
             tc.tile_pool(name="ps_d", bufs=1, space="PSUM") as ps_d, \
             tc.tile_pool(name="ps_f", bufs=2, space="PSUM") as ps_f:

            ONEC = ONESB[:, 0:1]
            ONERF = ONESF[0:1, :]

            def emit_norm(pend):
                ph, pqb, ou, den_sb = pend
                rd = pnrm.tile([1, SBLK], F32, name="rd")
                nc.vector.reciprocal_approx_fast(out=rd, in_=den_sb)
                rdb = pnrm.tile([1, SBLK], BF16, name="rdb")
                nc.vector.tensor_copy(rdb, rd)
                psb = ps_f.tile([P, SBLK], F32, name="pf")
                nc.tensor.matmul(psb, lhsT=ONESB[0:1, :], rhs=rdb,
                                 start=True, stop=True)
                rbc = pnrm.tile([P, SBLK], BF16, name="rbc")
                nc.vector.tensor_copy(rbc, psb)
                pqsl = slice(pqb * SBLK, (pqb + 1) * SBLK)
                for dvc in range(2):
                    nc.vector.tensor_mul(OUTN[:, 2 * ph + dvc, pqsl],
                                         ou[dvc], rbc)

            def emit_oproj_quarter(qb):
                for st in range(4 * qb, 4 * qb + 4):
                    stsl = slice(st * P, (st + 1) * P)
                    for nb in range(NSBLK):
                        psf = ps_f.tile([P, SBLK], F32, name="pf")
                        for dvc in range(NQCH):
                            nc.tensor.matmul(psf, lhsT=OUTN[:, dvc, stsl],
                                             rhs=WO[:, dvc, nb * SBLK:(nb + 1) * SBLK],
                                             start=(dvc == 0), stop=(dvc == NQCH - 1))
                        fsb = pfin.tile([P, SBLK], F32, name="fsb")
                        if (st + nb) % 2 == 0:
                            nc.vector.tensor_copy(fsb, psf)
                        else:
                            nc.scalar.copy(fsb, psf)
                        nc.sync.dma_start(out=outp[stsl, nb * SBLK:(nb + 1) * SBLK],
                                          in_=fsb)

            pending = None          # (h, qb, ou, den_sb) awaiting norm
            pending_oproj = None    # qb awaiting o_proj emission
            for h in range(HEADS_PER_CORE):
                for qb in range(NSBLK):
                    qsl = slice(qb * SBLK, (qb + 1) * SBLK)
                    klim = 4 * (qb + 1) if causal else NKC
                    pso = [ps_o.tile([P, SBLK], F32, name="po") for _ in range(2)]
                    pden = ps_d.tile([1, SBLK], F32, name="pd")
                    exs = [None] * klim

                    # diag tiles only need q >= k: trim their q-range to
                    # [delta, 512) (the causal staircase handles the rest)
                    def qoff(i):
                        if causal and i >= 4 * qb:
                            return 128 * i - 512 * qb
                        return 0

                    widths = [SBLK - qoff(i) for i in range(klim)]

                    def emit_av(i):
                        ex, w = exs[i], widths[i]
                        for dvc in range(2):
                            nc.tensor.matmul(pso[dvc][:, SBLK - w:],
                                             lhsT=VN[:, i, dvc * P:(dvc + 1) * P],
                                             rhs=ex[:, :w], start=(i == 0),
                                             stop=(i == klim - 1))

                    for i in range(klim):
                        qo, w = qoff(i), widths[i]
                        pss = ps_s.tile([P, SBLK], F32, name="ps")
                        for c in range(NKCH):
                            nc.tensor.matmul(pss[:, :w],
                                             lhsT=KT[:, c, i * P:(i + 1) * P],
                                             rhs=QT[:, 2 * h + c,
                                                    qb * SBLK + qo:(qb + 1) * SBLK],
                                             start=(c == 0), stop=(c == NKCH - 1))
                        if i == 1 and pending is not None:
                            emit_norm(pending)
                            pending = None
                        ex = pexp.tile([P, SBLK], BF16, name="ex")
                        nc.scalar.activation(ex[:, :w], pss[:, :w], EXP,
                                             scale=1.0 / 16.0)
                        if causal and i >= 4 * qb:
                            nc.vector.tensor_mul(ex[:, :w], ex[:, :w],
                                                 STAIR[:, 512:512 + w])
                        if not causal:
                            mt = pmask.tile([P, SBLK], BF16, name="mt")
                            nc.sync.dma_start(out=mt,
                                              in_=maskT[i * P:(i + 1) * P, qsl])
                            nc.vector.tensor_mul(ex, ex, mt)
                        exs[i] = ex
                        if i == 3 and pending_oproj is not None:
                            emit_oproj_quarter(pending_oproj)
                            pending_oproj = None
                        if i >= 2:
                            emit_av(i - 2)
                    emit_av(klim - 2)
                    emit_av(klim - 1)
                    # batched denominator reduce: ONEC stays stationary, so
                    # these klim matmuls stream back-to-back with no
                    # LDWEIGHTS churn and no unsatisfied waits
                    for i in range(klim):
                        w = widths[i]
                        nc.tensor.matmul(pden[:, SBLK - w:], lhsT=ONEC,
                                         rhs=exs[i][:, :w],
                                         start=(i == 0), stop=(i == klim - 1))

                    # immediate drains: free pso/pden quickly
                    ou = [pou.tile([P, SBLK], BF16, name="ou") for _ in range(2)]
                    for dvc in range(2):
                        nc.vector.tensor_copy(ou[dvc], pso[dvc])
                    den_sb = pnrm.tile([1, SBLK], F32, name="dsb")
                    nc.scalar.copy(den_sb, pden)
                    pending = (h, qb, ou, den_sb)
                    if h == 1:
                        pending_oproj = qb
            emit_norm(pending)
            emit_oproj_quarter(pending_oproj)

    nc.finalize()
    return nc


def _get_nc(causal: bool):
    key = bool(causal)
    if key not in _BUILD_CACHE:
        _BUILD_CACHE[key] = _build(causal)
    return _BUILD_CACHE[key]


def _rope_tables(position_ids_b):
    # cosT/sinT: [HD, S] bf16, transposed layout for the [d, s] dataflow
    pos = np.asarray(position_ids_b, dtype=np.float64)
    inv = 1.0 / (ROPE_BASE ** (np.arange(0, HD, 2, dtype=np.float64) / HD))
    f = pos[:, None] * inv[None, :]            # [S, HD/2]
    emb = np.concatenate([f, f], axis=1)       # [S, HD]
    cosT = np.ascontiguousarray(np.cos(emb).T).astype(ml_dtypes.bfloat16)
    sinT = np.ascontiguousarray(np.sin(emb).T).astype(ml_dtypes.bfloat16)
    return cosT, sinT


def _is_causal(attention_mask):
    m = np.asarray(attention_mask)
    if m.shape != (B, 1, S, S):
        return False
    tri = np.tril(np.ones((S, S), dtype=bool))
    canon = np.where(tri, np.float32(0.0), np.float32(-1e9))
    return all(np.array_equal(m[b, 0], canon) for b in range(B))


_ONESB_NP = np.ones((P, P), dtype=ml_dtypes.bfloat16)
_ONESF_NP = np.ones((1, P), dtype=np.float32)


def _stair():
    # multiplicative staircase: stair[p, j] = 1 if (j - 512) >= p else 0
    j = np.arange(2 * SBLK)[None, :] - SBLK
    p = np.arange(P)[:, None]
    return np.where(j >= p, 1.0, 0.0).astype(ml_dtypes.bfloat16)


def kernel(hidden_state, attention_mask, position_ids, Wq, Wk, Wv, Wo,
           _trace=False, _tmpdir=None):
    global LAST_EXEC_TIME_NS
    hidden_state = np.asarray(hidden_state, dtype=np.float32)
    Wq = np.asarray(Wq, dtype=np.float32)
    Wk = np.asarray(Wk, dtype=np.float32)
    Wv = np.asarray(Wv, dtype=np.float32)
    Wo = np.asarray(Wo, dtype=np.float32)

    causal = _is_causal(attention_mask)
    nc = _get_nc(causal)

    stair = _stair() if causal else None
    wk_b = Wk.astype(ml_dtypes.bfloat16)
    wv_b = Wv.astype(ml_dtypes.bfloat16)
    per_batch = {}
    for b in range(B):
        hTb = np.ascontiguousarray(hidden_state[b].T).astype(ml_dtypes.bfloat16)
        cosTb, sinTb = _rope_tables(position_ids[b])
        mb = None
        if not causal:
            mb = np.ascontiguousarray(
                np.exp(np.asarray(attention_mask, dtype=np.float64)[b, 0].T)
            ).astype(ml_dtypes.bfloat16)
        per_batch[b] = (hTb, cosTb, sinTb, mb)

    in_maps = []
    for core in range(8):
        b = core // 4
        hp = core % 4
        hTb, cosTb, sinTb, mb = per_batch[b]
        im = {
            "hT": hTb,
            "onesb": _ONESB_NP,
            "onesf": _ONESF_NP,
            "wq": np.ascontiguousarray(
                Wq[:, hp * DQ:(hp + 1) * DQ]).astype(ml_dtypes.bfloat16),
            "wk": wk_b,
            "wv": wv_b,
            "wo": np.ascontiguousarray(
                Wo[hp * DQ:(hp + 1) * DQ, :]).astype(ml_dtypes.bfloat16),
            "cosT": cosTb,
            "sinT": sinTb,
        }
        if causal:
            im["stair"] = stair
        else:
            im["emaskT"] = mb
        in_maps.append(im)

    res = run_bass_kernel_spmd(nc, in_maps, core_ids=list(range(8)),
                               trace=_trace, tmpdir=_tmpdir)
    LAST_EXEC_TIME_NS = res.exec_time_ns

    out = np.empty((B, S, D), dtype=np.float32)
    for b in range(B):
        acc = res.results[4 * b]["out_partial"].astype(np.float32).copy()
        for hp in range(1, 4):
            acc += res.results[4 * b + hp]["out_partial"]
        out[b] = acc
    return out


# revision 16
# speedup vs baseline: 1.5832x; 1.0126x over previous
"""GemmaAttention (B=2, S=2048, D=2048, H=8, KV=1, HD=256) on 8 trn2 NeuronCores.

Sharding: DP=2 over batch x TP=4 over head-pairs. Core c handles batch c//4 and
heads {2*(c%4), 2*(c%4)+1}. Each core computes its partial o_proj output
(row-parallel Wo); the host sums the 4 partials per batch (the all-reduce is
folded into the host-side unshard).

All matmuls run in bf16 (fp32 PSUM accumulate): fp32r streams at ~half the
bf16 column rate on the PE (389ns vs 213ns per N=512 matmul), so bf16 nearly
doubles tensor-engine throughput. rel-err budget is 2e-2; bf16 lands ~3e-3.

Dataflow per core:
  phase A (projections, per 512-col s-block):
    QT[dq,s], KT[dk,s] accumulate c-major over 16 D-chunks (6 PSUM banks),
    drained via ACT copy (psum->bf16 SBUF, frees banks fast) then RoPE on DVE
    in bf16 (2x mode). V[s,dv] computed directly (lhsT = hT chunk) in 128-row
    chains on 2 more banks -- this PE work covers the RoPE drain latency so
    the PE never idles at s-block boundaries.
  phase C (attention, per (head, q-block) item, software-pipelined):
    scoresT[k,q] = KT_chunk.T @ QT with a skew-2 pipeline: S(i) issues two
    iterations ahead of DEN(i)/AV(i) so the ACT exp (+DVE causal staircase)
    latency is hidden behind PE work. Denominators accumulate on the PE
    (ONEC.T @ ex into a [1,512] PSUM row). Per-item normalization is deferred
    one item (reciprocal_approx_fast + ONES-broadcast matmul + DVE scale) and
    o_proj quarters are injected into the next item's PE stream.
"""

import numpy as np
import ml_dtypes

import concourse.bass as bass
import concourse.tile as tile
import concourse.mybir as mybir
from concourse import bacc
from concourse.bass_utils import run_bass_kernel_spmd

P = 128
B, S, D = 2, 2048, 2048
H, KV, HD = 8, 1, 256
ROPE_BASE = 10000.0

HEADS_PER_CORE = 2
DQ = HEADS_PER_CORE * HD          # 512 q-dims per core
DCH = D // P                      # 16 contraction chunks
SBLK = 512                        # s-tile for projection rhs / q-tile
NSBLK = S // SBLK                 # 4
NKC = S // P                      # 16 key chunks
NQCH = DQ // P                    # 4 QT partition chunks
NKCH = HD // P                    # 2 KT partition chunks

F32 = mybir.dt.float32
BF16 = mybir.dt.bfloat16
EXP = mybir.ActivationFunctionType.Exp

LAST_EXEC_TIME_NS = None

_BUILD_CACHE = {}


def _build(causal: bool):
    nc = bacc.Bacc()

    hT = nc.declare_dram_parameter("hT", [D, S], BF16, isOutput=False)
    wq = nc.declare_dram_parameter("wq", [D, DQ], BF16, isOutput=False)
    wk = nc.declare_dram_parameter("wk", [D, HD], BF16, isOutput=False)
    wv = nc.declare_dram_parameter("wv", [D, HD], BF16, isOutput=False)
    wo = nc.declare_dram_parameter("wo", [DQ, D], BF16, isOutput=False)
    cosT = nc.declare_dram_parameter("cosT", [HD, S], BF16, isOutput=False)
    sinT = nc.declare_dram_parameter("sinT", [HD, S], BF16, isOutput=False)
    onesb = nc.declare_dram_parameter("onesb", [P, P], BF16, isOutput=False)
    sel4 = nc.declare_dram_parameter("sel4", [P, P], BF16, isOutput=False)
    if causal:
        stair = nc.declare_dram_parameter("stair", [P, 2 * SBLK], BF16,
                                          isOutput=False)
    else:
        maskT = nc.declare_dram_parameter("emaskT", [S, S], BF16, isOutput=False)
    outp = nc.declare_dram_parameter("out_partial", [S, D], F32, isOutput=True)

    from contextlib import ExitStack
    with tile.TileContext(nc) as tc, ExitStack() as ctx:
        pq = ctx.enter_context(tc.tile_pool(name="pq", bufs=1))
        QT = pq.tile([P, NQCH, S], BF16, name="QT")
        KT = pq.tile([P, NKCH, S], BF16, name="KT")
        VN = pq.tile([P, NKC, HD], BF16, name="VN")
        OUTN = pq.tile([P, NQCH, S], BF16, name="OUTN")
        WO = pq.tile([P, NQCH, D], BF16, name="WO")
        ONESB = pq.tile([P, P], BF16, name="ONESB")
        SEL4 = pq.tile([P, P], BF16, name="SEL4")
        if causal:
            STAIR = pq.tile([P, 2 * SBLK], BF16, name="STAIR")

        # ---- phase A: projections + RoPE -----------------------------
        with tc.tile_pool(name="pw", bufs=1) as pw, \
             tc.tile_pool(name="pht", bufs=16) as pht, \
             tc.tile_pool(name="pcs", bufs=4) as pcs, \
             tc.tile_pool(name="pqt", bufs=8) as pqt, \
             tc.tile_pool(name="ptmp", bufs=6) as ptmp, \
             tc.tile_pool(name="ppqk", bufs=6, space="PSUM") as ppqk, \
             tc.tile_pool(name="ppv", bufs=2, space="PSUM") as ppv:
            WQ = pw.tile([P, DCH, DQ], BF16, name="WQ")
            WK = pw.tile([P, DCH, HD], BF16, name="WK")
            WV = pw.tile([P, DCH, HD], BF16, name="WV")

            hts = []
            for sb in range(NSBLK):
                ssl = slice(sb * SBLK, (sb + 1) * SBLK)
                if sb == 0:
                    # hT loads in two stages: the sb0 columns stream
                    # chunk-by-chunk with the weights (demand order), the
                    # rest (3KB lines) follows and makes sblocks 1-3 fully
                    # DMA-free
                    for c in range(DCH):
                        ht = pht.tile([P, S], BF16, name="ht")
                        hts.append(ht)
                        nc.sync.dma_start(out=ht[:, 0:SBLK],
                                          in_=hT[c * P:(c + 1) * P, 0:SBLK])
                        nc.sync.dma_start(out=WQ[:, c, :], in_=wq[c * P:(c + 1) * P, :])
                        nc.sync.dma_start(out=WK[:, c, :], in_=wk[c * P:(c + 1) * P, :])
                        nc.sync.dma_start(out=WV[:, c, :], in_=wv[c * P:(c + 1) * P, :])
                        if c == 0:
                            nc.sync.dma_start(out=ONESB, in_=onesb[:, :])
                            nc.sync.dma_start(out=SEL4, in_=sel4[:, :])
                            if causal:
                                nc.sync.dma_start(out=STAIR, in_=stair[:, :])
                if sb == NSBLK - 1:
                    for cc in range(NQCH):
                        # o_proj weights are first needed deep into phase C;
                        # stream them late so they never delay hT/cos/sin
                        nc.sync.dma_start(out=WO[:, cc, :],
                                          in_=wo[cc * P:(cc + 1) * P, :])
                COSb = pcs.tile([P, NKCH, SBLK], BF16, name="cosb")
                SINb = pcs.tile([P, NKCH, SBLK], BF16, name="sinb")
                for jj in range(NKCH):
                    nc.sync.dma_start(out=COSb[:, jj, :],
                                      in_=cosT[jj * P:(jj + 1) * P, ssl])
                    nc.sync.dma_start(out=SINb[:, jj, :],
                                      in_=sinT[jj * P:(jj + 1) * P, ssl])
                if sb == 0:
                    for c in range(DCH):
                        nc.sync.dma_start(out=hts[c][:, SBLK:],
                                          in_=hT[c * P:(c + 1) * P, SBLK:])

                # Q/K accumulation, c-major (DMA-friendly: each ht used
                # 6x right after it lands)
                psq = [ppqk.tile([P, SBLK], F32, name="pp") for _ in range(NQCH)]
                psk = [ppqk.tile([P, SBLK], F32, name="pp") for _ in range(NKCH)]
                for c in range(DCH):
                    for i in range(NQCH):
                        nc.tensor.matmul(psq[i], lhsT=WQ[:, c, i * P:(i + 1) * P],
                                         rhs=hts[c][:, ssl], start=(c == 0),
                                         stop=(c == DCH - 1))
                    for j in range(NKCH):
                        nc.tensor.matmul(psk[j], lhsT=WK[:, c, j * P:(j + 1) * P],
                                         rhs=hts[c][:, ssl], start=(c == 0),
                                         stop=(c == DCH - 1))
                # fast ACT drains free the 6 banks; RoPE runs on DVE from
                # bf16 SBUF copies (2x mode) off the PE critical path
                qts = []
                for ps in psq + psk:
                    t = pqt.tile([P, SBLK], BF16, name="qt")
                    nc.scalar.copy(t, ps)
                    qts.append(t)

                def rope_pair(b0, b1, out0, out1):
                    c0 = COSb[:, 0, :]; c1 = COSb[:, 1, :]
                    s0 = SINb[:, 0, :]; s1 = SINb[:, 1, :]
                    t1 = ptmp.tile([P, SBLK], BF16, name="t")
                    t2 = ptmp.tile([P, SBLK], BF16, name="t")
                    nc.vector.tensor_mul(t1, b0, c0)
                    nc.vector.tensor_mul(t2, b1, s0)
                    nc.vector.tensor_sub(out0, t1, t2)
                    t3 = ptmp.tile([P, SBLK], BF16, name="t")
                    t4 = ptmp.tile([P, SBLK], BF16, name="t")
                    nc.vector.tensor_mul(t3, b1, c1)
                    nc.vector.tensor_mul(t4, b0, s1)
                    nc.vector.tensor_add(out1, t3, t4)

                for h in range(HEADS_PER_CORE):
                    rope_pair(qts[2 * h], qts[2 * h + 1],
                              QT[:, 2 * h, ssl], QT[:, 2 * h + 1, ssl])
                rope_pair(qts[NQCH], qts[NQCH + 1],
                          KT[:, 0, ssl], KT[:, 1, ssl])

                # direct V chains (PE work that covers the drains above)
                for si in range(SBLK // P):
                    # full-bank tile ([P,SBLK] f32 = 2KB) so two ppv bufs can
                    # never share a PSUM bank (PE-write + DVE-read collision)
                    psv = ppv.tile([P, SBLK], F32, name="pv")
                    for c in range(DCH):
                        nc.tensor.matmul(psv[:, :HD],
                                         lhsT=hts[c][:, sb * SBLK + si * P:
                                                     sb * SBLK + (si + 1) * P],
                                         rhs=WV[:, c, :], start=(c == 0),
                                         stop=(c == DCH - 1))
                    nc.vector.tensor_copy(VN[:, sb * (SBLK // P) + si, :],
                                          psv[:, :HD])

        # ---- phase C+D: attention + norm + interleaved o_proj ---------
        with tc.tile_pool(name="pexp", bufs=17) as pexp, \
             tc.tile_pool(name="pou", bufs=4) as pou, \
             tc.tile_pool(name="pnrm", bufs=6) as pnrm, \
             tc.tile_pool(name="pfin", bufs=4) as pfin, \
             tc.tile_pool(name="pmask", bufs=4) as pmask, \
             tc.tile_pool(name="ps_s", bufs=3, space="PSUM") as ps_s, \
             tc.tile_pool(name="ps_o", bufs=2, space="PSUM") as ps_o, \
             tc.tile_pool(name="ps_d", bufs=1, space="PSUM") as ps_d, \
             tc.tile_pool(name="ps_f", bufs=2, space="PSUM") as ps_f:

            ONEC = ONESB[:, 0:1]
            # denominator accumulator: one full PSUM bank, row 0 holds the
            # running sum; other partitions are zeroed once so the full-bank
            # bf16 drain never captures inf/nan junk (SEL row-0 matmul
            # broadcasts row 0 and zeroes the rest)
            pden = ps_d.tile([P, SBLK], F32, name="pd")
            nc.vector.memset(pden, 0.0)

            def emit_norm(pend):
                ph, pqb, ou, den128 = pend
                # psb[m, q] = sum of the 4 packed den rows, broadcast to all
                # 128 partitions in a single matmul (SEL4 row m is 1 on
                # partitions {0,32,64,96})
                psb = ps_f.tile([P, SBLK], F32, name="pf")
                nc.tensor.matmul(psb, lhsT=SEL4, rhs=den128,
                                 start=True, stop=True)
                rbcf = pnrm.tile([P, SBLK], F32, name="rbc")
                nc.vector.reciprocal_approx_fast(out=rbcf, in_=psb)
                pqsl = slice(pqb * SBLK, (pqb + 1) * SBLK)
                for dvc in range(2):
                    nc.vector.tensor_mul(OUTN[:, 2 * ph + dvc, pqsl],
                                         ou[dvc], rbcf)

            def emit_oproj_quarter(qb):
                for st in range(4 * qb, 4 * qb + 4):
                    stsl = slice(st * P, (st + 1) * P)
                    for nb in range(NSBLK):
                        psf = ps_f.tile([P, SBLK], F32, name="pf")
                        for dvc in range(NQCH):
                            nc.tensor.matmul(psf, lhsT=OUTN[:, dvc, stsl],
                                             rhs=WO[:, dvc, nb * SBLK:(nb + 1) * SBLK],
                                             start=(dvc == 0), stop=(dvc == NQCH - 1))
                        fsb = pfin.tile([P, SBLK], F32, name="fsb")
                        if (st + nb) % 2 == 0:
                            nc.vector.tensor_copy(fsb, psf)
                        else:
                            nc.scalar.copy(fsb, psf)
                        nc.sync.dma_start(out=outp[stsl, nb * SBLK:(nb + 1) * SBLK],
                                          in_=fsb)

            pending = None          # (h, qb, ou, den_sb) awaiting norm
            pending_oproj = None    # qb awaiting o_proj emission
            for h in range(HEADS_PER_CORE):
                for qb in range(NSBLK):
                    qsl = slice(qb * SBLK, (qb + 1) * SBLK)
                    klim = 4 * (qb + 1) if causal else NKC
                    pso = [ps_o.tile([P, SBLK], F32, name="po") for _ in range(2)]
                    exs = [None] * klim

                    # diag tiles only need q >= k: trim their q-range to
                    # [delta, 512) (the causal staircase handles the rest)
                    def qoff(i):
                        if causal and i >= 4 * qb:
                            return 128 * i - 512 * qb
                        return 0

                    widths = [SBLK - qoff(i) for i in range(klim)]

                    def emit_av(i):
                        ex, w = exs[i], widths[i]
                        for dvc in range(2):
                            nc.tensor.matmul(pso[dvc][:, SBLK - w:],
                                             lhsT=VN[:, i, dvc * P:(dvc + 1) * P],
                                             rhs=ex[:, :w], start=(i == 0),
                                             stop=(i == klim - 1))

                    for i in range(klim):
                        qo, w = qoff(i), widths[i]
                        pss = ps_s.tile([P, SBLK], F32, name="ps")
                        for c in range(NKCH):
                            nc.tensor.matmul(pss[:, :w],
                                             lhsT=KT[:, c, i * P:(i + 1) * P],
                                             rhs=QT[:, 2 * h + c,
                                                    qb * SBLK + qo:(qb + 1) * SBLK],
                                             start=(c == 0), stop=(c == NKCH - 1))
                        if i == 1 and pending is not None:
                            emit_norm(pending)
                            pending = None
                        ex = pexp.tile([P, SBLK], BF16, name="ex")
                        nc.scalar.activation(ex[:, :w], pss[:, :w], EXP,
                                             scale=1.0 / 16.0)
                        if causal and i >= 4 * qb:
                            nc.vector.tensor_mul(ex[:, :w], ex[:, :w],
                                                 STAIR[:, 512:512 + w])
                        if not causal:
                            mt = pmask.tile([P, SBLK], BF16, name="mt")
                            nc.sync.dma_start(out=mt,
                                              in_=maskT[i * P:(i + 1) * P, qsl])
                            nc.vector.tensor_mul(ex, ex, mt)
                        exs[i] = ex
                        if i == 3 and pending_oproj is not None:
                            emit_oproj_quarter(pending_oproj)
                            pending_oproj = None
                        if i >= 2:
                            emit_av(i - 2)
                    emit_av(klim - 2)
                    emit_av(klim - 1)
                    # batched denominator reduce: ONEC stays stationary, so
                    # these klim matmuls stream back-to-back with no
                    # LDWEIGHTS churn and no unsatisfied waits
                    for i in range(klim):
                        w = widths[i]
                        nc.tensor.matmul(pden[0:1, SBLK - w:],
                                         lhsT=ONEC, rhs=exs[i][:, :w],
                                         start=(i == 0), stop=(i == klim - 1))

                    # immediate drains: free pso/pden quickly
                    ou = [pou.tile([P, SBLK], BF16, name="ou") for _ in range(2)]
                    for dvc in range(2):
                        nc.vector.tensor_copy(ou[dvc], pso[dvc])
                    den128 = pnrm.tile([P, SBLK], BF16, name="dsb")
                    nc.scalar.copy(den128, pden)
                    pending = (h, qb, ou, den128)
                    if h == 1:
                        pending_oproj = qb
            emit_norm(pending)
            emit_oproj_quarter(pending_oproj)

    nc.finalize()
    return nc


def _get_nc(causal: bool):
    key = bool(causal)
    if key not in _BUILD_CACHE:
        _BUILD_CACHE[key] = _build(causal)
    return _BUILD_CACHE[key]


def _rope_tables(position_ids_b):
    # cosT/sinT: [HD, S] bf16, transposed layout for the [d, s] dataflow
    pos = np.asarray(position_ids_b, dtype=np.float64)
    inv = 1.0 / (ROPE_BASE ** (np.arange(0, HD, 2, dtype=np.float64) / HD))
    f = pos[:, None] * inv[None, :]            # [S, HD/2]
    emb = np.concatenate([f, f], axis=1)       # [S, HD]
    cosT = np.ascontiguousarray(np.cos(emb).T).astype(ml_dtypes.bfloat16)
    sinT = np.ascontiguousarray(np.sin(emb).T).astype(ml_dtypes.bfloat16)
    return cosT, sinT


def _is_causal(attention_mask):
    m = np.asarray(attention_mask)
    if m.shape != (B, 1, S, S):
        return False
    tri = np.tril(np.ones((S, S), dtype=bool))
    canon = np.where(tri, np.float32(0.0), np.float32(-1e9))
    return all(np.array_equal(m[b, 0], canon) for b in range(B))


_ONESB_NP = np.ones((P, P), dtype=ml_dtypes.bfloat16)
_SEL4_NP = np.zeros((P, P), dtype=ml_dtypes.bfloat16)
_SEL4_NP[0, :] = 1.0


def _stair():
    # multiplicative staircase: stair[p, j] = 1 if (j - 512) >= p else 0
    j = np.arange(2 * SBLK)[None, :] - SBLK
    p = np.arange(P)[:, None]
    return np.where(j >= p, 1.0, 0.0).astype(ml_dtypes.bfloat16)


def kernel(hidden_state, attention_mask, position_ids, Wq, Wk, Wv, Wo,
           _trace=False, _tmpdir=None):
    global LAST_EXEC_TIME_NS
    hidden_state = np.asarray(hidden_state, dtype=np.float32)
    Wq = np.asarray(Wq, dtype=np.float32)
    Wk = np.asarray(Wk, dtype=np.float32)
    Wv = np.asarray(Wv, dtype=np.float32)
    Wo = np.asarray(Wo, dtype=np.float32)

    causal = _is_causal(attention_mask)
    nc = _get_nc(causal)

    stair = _stair() if causal else None
    wk_b = Wk.astype(ml_dtypes.bfloat16)
    wv_b = Wv.astype(ml_dtypes.bfloat16)
    per_batch = {}
    for b in range(B):
        hTb = np.ascontiguousarray(hidden_state[b].T).astype(ml_dtypes.bfloat16)
        cosTb, sinTb = _rope_tables(position_ids[b])
        mb = None
        if not causal:
            mb = np.ascontiguousarray(
                np.exp(np.asarray(attention_mask, dtype=np.float64)[b, 0].T)
            ).astype(ml_dtypes.bfloat16)
        per_batch[b] = (hTb, cosTb, sinTb, mb)

    in_maps = []
    for core in range(8):
        b = core // 4
        hp = core % 4
        hTb, cosTb, sinTb, mb = per_batch[b]
        im = {
            "hT": hTb,
            "onesb": _ONESB_NP,
            "sel4": _SEL4_NP,
            "wq": np.ascontiguousarray(
                Wq[:, hp * DQ:(hp + 1) * DQ]).astype(ml_dtypes.bfloat16),
            "wk": wk_b,
            "wv": wv_b,
            "wo": np.ascontiguousarray(
                Wo[hp * DQ:(hp + 1) * DQ, :]).astype(ml_dtypes.bfloat16),
            "cosT": cosTb,
            "sinT": sinTb,
        }
        if causal:
            im["stair"] = stair
        else:
            im["emaskT"] = mb
        in_maps.append(im)

    res = run_bass_kernel_spmd(nc, in_maps, core_ids=list(range(8)),
                               trace=_trace, tmpdir=_tmpdir)
    LAST_EXEC_TIME_NS = res.exec_time_ns

    out = np.empty((B, S, D), dtype=np.float32)
    for b in range(B):
        acc = res.results[4 * b]["out_partial"].astype(np.float32).copy()
        for hp in range(1, 4):
            acc += res.results[4 * b + hp]["out_partial"]
        out[b] = acc
    return out


# revision 17
# speedup vs baseline: 1.6847x; 1.0641x over previous
"""GemmaAttention (B=2, S=2048, D=2048, H=8, KV=1, HD=256) on 8 trn2 NeuronCores.

Sharding: DP=2 over batch x TP=4 over head-pairs. Core c handles batch c//4 and
heads {2*(c%4), 2*(c%4)+1}. Each core computes its partial o_proj output
(row-parallel Wo); the host sums the 4 partials per batch (the all-reduce is
folded into the host-side unshard).

All matmuls run in bf16 (fp32 PSUM accumulate): fp32r streams at ~half the
bf16 column rate on the PE (389ns vs 213ns per N=512 matmul), so bf16 nearly
doubles tensor-engine throughput. rel-err budget is 2e-2; bf16 lands ~3e-3.

Dataflow per core:
  phase A (projections, per 512-col s-block):
    QT[dq,s], KT[dk,s] accumulate c-major over 16 D-chunks (6 PSUM banks),
    drained via ACT copy (psum->bf16 SBUF, frees banks fast) then RoPE on DVE
    in bf16 (2x mode). V[s,dv] computed directly (lhsT = hT chunk) in 128-row
    chains on 2 more banks -- this PE work covers the RoPE drain latency so
    the PE never idles at s-block boundaries.
  phase C (attention, per (head, q-block) item, software-pipelined):
    scoresT[k,q] = KT_chunk.T @ QT with a skew-2 pipeline: S(i) issues two
    iterations ahead of DEN(i)/AV(i) so the ACT exp (+DVE causal staircase)
    latency is hidden behind PE work. Denominators accumulate on the PE
    (ONEC.T @ ex into a [1,512] PSUM row). Per-item normalization is deferred
    one item (reciprocal_approx_fast + ONES-broadcast matmul + DVE scale) and
    o_proj quarters are injected into the next item's PE stream.
"""

import numpy as np
import ml_dtypes

import concourse.bass as bass
import concourse.tile as tile
import concourse.mybir as mybir
from concourse import bacc
from concourse.bass_utils import run_bass_kernel_spmd

P = 128
B, S, D = 2, 2048, 2048
H, KV, HD = 8, 1, 256
ROPE_BASE = 10000.0

HEADS_PER_CORE = 2
DQ = HEADS_PER_CORE * HD          # 512 q-dims per core
DCH = D // P                      # 16 contraction chunks
SBLK = 512                        # s-tile for projection rhs / q-tile
NSBLK = S // SBLK                 # 4
NKC = S // P                      # 16 key chunks
NQCH = DQ // P                    # 4 QT partition chunks
NKCH = HD // P                    # 2 KT partition chunks

F32 = mybir.dt.float32
BF16 = mybir.dt.bfloat16
EXP = mybir.ActivationFunctionType.Exp

LAST_EXEC_TIME_NS = None

_BUILD_CACHE = {}


def _build(causal: bool):
    nc = bacc.Bacc()

    # packed layouts: [..., c, :] is chunk c's 128-partition tile slice, so
    # grouped DMAs read >=2KB contiguous per partition (1KB lines cap the
    # DMA ring at ~150 GB/s; 2-3KB lines reach ~300-390 GB/s)
    hTp0 = nc.declare_dram_parameter("hTp0", [P, DCH, SBLK], BF16, isOutput=False)
    hTp1 = nc.declare_dram_parameter("hTp1", [P, DCH, S - SBLK], BF16,
                                     isOutput=False)
    wqp = nc.declare_dram_parameter("wqp", [P, DCH, DQ], BF16, isOutput=False)
    wkp = nc.declare_dram_parameter("wkp", [P, DCH, HD], BF16, isOutput=False)
    wvp = nc.declare_dram_parameter("wvp", [P, DCH, HD], BF16, isOutput=False)
    wo = nc.declare_dram_parameter("wo", [DQ, D], BF16, isOutput=False)
    cosT = nc.declare_dram_parameter("cosT", [HD, S], BF16, isOutput=False)
    sinT = nc.declare_dram_parameter("sinT", [HD, S], BF16, isOutput=False)
    onesb = nc.declare_dram_parameter("onesb", [P, P], BF16, isOutput=False)
    sel4 = nc.declare_dram_parameter("sel4", [P, P], BF16, isOutput=False)
    if causal:
        stair = nc.declare_dram_parameter("stair", [P, 2 * SBLK], BF16,
                                          isOutput=False)
    else:
        maskT = nc.declare_dram_parameter("emaskT", [S, S], BF16, isOutput=False)
    outp = nc.declare_dram_parameter("out_partial", [S, D], F32, isOutput=True)

    from contextlib import ExitStack
    with tile.TileContext(nc) as tc, ExitStack() as ctx:
        pq = ctx.enter_context(tc.tile_pool(name="pq", bufs=1))
        QT = pq.tile([P, NQCH, S], BF16, name="QT")
        KT = pq.tile([P, NKCH, S], BF16, name="KT")
        VN = pq.tile([P, NKC, HD], BF16, name="VN")
        OUTN = pq.tile([P, NQCH, S], BF16, name="OUTN")
        WO = pq.tile([P, NQCH, D], BF16, name="WO")
        ONESB = pq.tile([P, P], BF16, name="ONESB")
        SEL4 = pq.tile([P, P], BF16, name="SEL4")
        if causal:
            STAIR = pq.tile([P, 2 * SBLK], BF16, name="STAIR")

        # ---- phase A: projections + RoPE -----------------------------
        with tc.tile_pool(name="pw", bufs=1) as pw, \
             tc.tile_pool(name="pht", bufs=1) as pht, \
             tc.tile_pool(name="pcs", bufs=4) as pcs, \
             tc.tile_pool(name="pqt", bufs=8) as pqt, \
             tc.tile_pool(name="ptmp", bufs=6) as ptmp, \
             tc.tile_pool(name="ppqk", bufs=6, space="PSUM") as ppqk, \
             tc.tile_pool(name="ppv", bufs=2, space="PSUM") as ppv:
            WQ = pw.tile([P, DCH, DQ], BF16, name="WQ")
            WK = pw.tile([P, DCH, HD], BF16, name="WK")
            WV = pw.tile([P, DCH, HD], BF16, name="WV")

            HTS = pht.tile([P, DCH, S], BF16, name="ht")
            for sb in range(NSBLK):
                ssl = slice(sb * SBLK, (sb + 1) * SBLK)
                if sb == 0:
                    # hT loads in two stages: the sb0 columns stream in
                    # 2-chunk groups (2KB lines) interleaved with the weight
                    # groups in demand order; the remaining columns follow
                    # in 12KB-line groups and make sblocks 1-3 fully DMA-free
                    nc.sync.dma_start(out=ONESB, in_=onesb[:, :])
                    nc.sync.dma_start(out=SEL4, in_=sel4[:, :])
                    if causal:
                        nc.sync.dma_start(out=STAIR, in_=stair[:, :])
                    for g in range(DCH // 2):
                        gs = slice(2 * g, 2 * g + 2)
                        nc.sync.dma_start(out=HTS[:, gs, 0:SBLK],
                                          in_=hTp0[:, gs, :])
                        nc.sync.dma_start(out=WQ[:, gs, :], in_=wqp[:, gs, :])
                        if g < 4:
                            g4 = slice(4 * g, 4 * g + 4)
                            nc.sync.dma_start(out=WK[:, g4, :], in_=wkp[:, g4, :])
                            nc.sync.dma_start(out=WV[:, g4, :], in_=wvp[:, g4, :])
                if sb == NSBLK - 1:
                    for cc in range(NQCH):
                        # o_proj weights are first needed deep into phase C;
                        # stream them late so they never delay hT/cos/sin
                        nc.sync.dma_start(out=WO[:, cc, :],
                                          in_=wo[cc * P:(cc + 1) * P, :])
                COSb = pcs.tile([P, NKCH, SBLK], BF16, name="cosb")
                SINb = pcs.tile([P, NKCH, SBLK], BF16, name="sinb")
                for jj in range(NKCH):
                    nc.sync.dma_start(out=COSb[:, jj, :],
                                      in_=cosT[jj * P:(jj + 1) * P, ssl])
                    nc.sync.dma_start(out=SINb[:, jj, :],
                                      in_=sinT[jj * P:(jj + 1) * P, ssl])
                if sb == 0:
                    for g in range(DCH // 4):
                        g4 = slice(4 * g, 4 * g + 4)
                        nc.sync.dma_start(out=HTS[:, g4, SBLK:],
                                          in_=hTp1[:, g4, :])

                # Q/K accumulation, c-major (DMA-friendly: each ht used
                # 6x right after it lands)
                psq = [ppqk.tile([P, SBLK], F32, name="pp") for _ in range(NQCH)]
                psk = [ppqk.tile([P, SBLK], F32, name="pp") for _ in range(NKCH)]
                for c in range(DCH):
                    for i in range(NQCH):
                        nc.tensor.matmul(psq[i], lhsT=WQ[:, c, i * P:(i + 1) * P],
                                         rhs=HTS[:, c, ssl], start=(c == 0),
                                         stop=(c == DCH - 1))
                    for j in range(NKCH):
                        nc.tensor.matmul(psk[j], lhsT=WK[:, c, j * P:(j + 1) * P],
                                         rhs=HTS[:, c, ssl], start=(c == 0),
                                         stop=(c == DCH - 1))
                # fast ACT drains free the 6 banks; RoPE runs on DVE from
                # bf16 SBUF copies (2x mode) off the PE critical path
                qts = []
                for ps in psq + psk:
                    t = pqt.tile([P, SBLK], BF16, name="qt")
                    nc.scalar.copy(t, ps)
                    qts.append(t)

                def rope_pair(b0, b1, out0, out1):
                    c0 = COSb[:, 0, :]; c1 = COSb[:, 1, :]
                    s0 = SINb[:, 0, :]; s1 = SINb[:, 1, :]
                    t1 = ptmp.tile([P, SBLK], BF16, name="t")
                    t2 = ptmp.tile([P, SBLK], BF16, name="t")
                    nc.vector.tensor_mul(t1, b0, c0)
                    nc.vector.tensor_mul(t2, b1, s0)
                    nc.vector.tensor_sub(out0, t1, t2)
                    t3 = ptmp.tile([P, SBLK], BF16, name="t")
                    t4 = ptmp.tile([P, SBLK], BF16, name="t")
                    nc.vector.tensor_mul(t3, b1, c1)
                    nc.vector.tensor_mul(t4, b0, s1)
                    nc.vector.tensor_add(out1, t3, t4)

                for h in range(HEADS_PER_CORE):
                    rope_pair(qts[2 * h], qts[2 * h + 1],
                              QT[:, 2 * h, ssl], QT[:, 2 * h + 1, ssl])
                rope_pair(qts[NQCH], qts[NQCH + 1],
                          KT[:, 0, ssl], KT[:, 1, ssl])

                # direct V chains (PE work that covers the drains above)
                for si in range(SBLK // P):
                    # full-bank tile ([P,SBLK] f32 = 2KB) so two ppv bufs can
                    # never share a PSUM bank (PE-write + DVE-read collision)
                    psv = ppv.tile([P, SBLK], F32, name="pv")
                    for c in range(DCH):
                        nc.tensor.matmul(psv[:, :HD],
                                         lhsT=HTS[:, c, sb * SBLK + si * P:
                                                  sb * SBLK + (si + 1) * P],
                                         rhs=WV[:, c, :], start=(c == 0),
                                         stop=(c == DCH - 1))
                    nc.vector.tensor_copy(VN[:, sb * (SBLK // P) + si, :],
                                          psv[:, :HD])

        # ---- phase C+D: attention + norm + interleaved o_proj ---------
        with tc.tile_pool(name="pexp", bufs=17) as pexp, \
             tc.tile_pool(name="pou", bufs=4) as pou, \
             tc.tile_pool(name="pnrm", bufs=6) as pnrm, \
             tc.tile_pool(name="pfin", bufs=4) as pfin, \
             tc.tile_pool(name="pmask", bufs=4) as pmask, \
             tc.tile_pool(name="ps_s", bufs=3, space="PSUM") as ps_s, \
             tc.tile_pool(name="ps_o", bufs=2, space="PSUM") as ps_o, \
             tc.tile_pool(name="ps_d", bufs=1, space="PSUM") as ps_d, \
             tc.tile_pool(name="ps_f", bufs=2, space="PSUM") as ps_f:

            ONEC = ONESB[:, 0:1]
            # denominator accumulator: one full PSUM bank, row 0 holds the
            # running sum; other partitions are zeroed once so the full-bank
            # bf16 drain never captures inf/nan junk (SEL row-0 matmul
            # broadcasts row 0 and zeroes the rest)
            pden = ps_d.tile([P, SBLK], F32, name="pd")
            nc.vector.memset(pden, 0.0)

            def emit_norm(pend):
                ph, pqb, ou, den128 = pend
                # psb[m, q] = sum of the 4 packed den rows, broadcast to all
                # 128 partitions in a single matmul (SEL4 row m is 1 on
                # partitions {0,32,64,96})
                psb = ps_f.tile([P, SBLK], F32, name="pf")
                nc.tensor.matmul(psb, lhsT=SEL4, rhs=den128,
                                 start=True, stop=True)
                rbcf = pnrm.tile([P, SBLK], F32, name="rbc")
                nc.vector.reciprocal_approx_fast(out=rbcf, in_=psb)
                pqsl = slice(pqb * SBLK, (pqb + 1) * SBLK)
                for dvc in range(2):
                    nc.vector.tensor_mul(OUTN[:, 2 * ph + dvc, pqsl],
                                         ou[dvc], rbcf)

            def emit_oproj_quarter(qb):
                for st in range(4 * qb, 4 * qb + 4):
                    stsl = slice(st * P, (st + 1) * P)
                    for nb in range(NSBLK):
                        psf = ps_f.tile([P, SBLK], F32, name="pf")
                        for dvc in range(NQCH):
                            nc.tensor.matmul(psf, lhsT=OUTN[:, dvc, stsl],
                                             rhs=WO[:, dvc, nb * SBLK:(nb + 1) * SBLK],
                                             start=(dvc == 0), stop=(dvc == NQCH - 1))
                        fsb = pfin.tile([P, SBLK], F32, name="fsb")
                        if (st + nb) % 2 == 0:
                            nc.vector.tensor_copy(fsb, psf)
                        else:
                            nc.scalar.copy(fsb, psf)
                        nc.sync.dma_start(out=outp[stsl, nb * SBLK:(nb + 1) * SBLK],
                                          in_=fsb)

            pending = None          # (h, qb, ou, den_sb) awaiting norm
            pending_oproj = None    # qb awaiting o_proj emission
            for h in range(HEADS_PER_CORE):
                for qb in range(NSBLK):
                    qsl = slice(qb * SBLK, (qb + 1) * SBLK)
                    klim = 4 * (qb + 1) if causal else NKC
                    pso = [ps_o.tile([P, SBLK], F32, name="po") for _ in range(2)]
                    exs = [None] * klim

                    # diag tiles only need q >= k: trim their q-range to
                    # [delta, 512) (the causal staircase handles the rest)
                    def qoff(i):
                        if causal and i >= 4 * qb:
                            return 128 * i - 512 * qb
                        return 0

                    widths = [SBLK - qoff(i) for i in range(klim)]

                    def emit_av(i):
                        ex, w = exs[i], widths[i]
                        for dvc in range(2):
                            nc.tensor.matmul(pso[dvc][:, SBLK - w:],
                                             lhsT=VN[:, i, dvc * P:(dvc + 1) * P],
                                             rhs=ex[:, :w], start=(i == 0),
                                             stop=(i == klim - 1))

                    for i in range(klim):
                        qo, w = qoff(i), widths[i]
                        pss = ps_s.tile([P, SBLK], F32, name="ps")
                        for c in range(NKCH):
                            nc.tensor.matmul(pss[:, :w],
                                             lhsT=KT[:, c, i * P:(i + 1) * P],
                                             rhs=QT[:, 2 * h + c,
                                                    qb * SBLK + qo:(qb + 1) * SBLK],
                                             start=(c == 0), stop=(c == NKCH - 1))
                        if i == 1 and pending is not None:
                            emit_norm(pending)
                            pending = None
                        ex = pexp.tile([P, SBLK], BF16, name="ex")
                        nc.scalar.activation(ex[:, :w], pss[:, :w], EXP,
                                             scale=1.0 / 16.0)
                        if causal and i >= 4 * qb:
                            nc.vector.tensor_mul(ex[:, :w], ex[:, :w],
                                                 STAIR[:, 512:512 + w])
                        if not causal:
                            mt = pmask.tile([P, SBLK], BF16, name="mt")
                            nc.sync.dma_start(out=mt,
                                              in_=maskT[i * P:(i + 1) * P, qsl])
                            nc.vector.tensor_mul(ex, ex, mt)
                        exs[i] = ex
                        if i == 3 and pending_oproj is not None:
                            emit_oproj_quarter(pending_oproj)
                            pending_oproj = None
                        if i >= 2:
                            emit_av(i - 2)
                    emit_av(klim - 2)
                    emit_av(klim - 1)
                    # batched denominator reduce: ONEC stays stationary, so
                    # these klim matmuls stream back-to-back with no
                    # LDWEIGHTS churn and no unsatisfied waits
                    for i in range(klim):
                        w = widths[i]
                        nc.tensor.matmul(pden[0:1, SBLK - w:],
                                         lhsT=ONEC, rhs=exs[i][:, :w],
                                         start=(i == 0), stop=(i == klim - 1))

                    # immediate drains: free pso/pden quickly
                    ou = [pou.tile([P, SBLK], BF16, name="ou") for _ in range(2)]
                    for dvc in range(2):
                        nc.vector.tensor_copy(ou[dvc], pso[dvc])
                    den128 = pnrm.tile([P, SBLK], BF16, name="dsb")
                    nc.scalar.copy(den128, pden)
                    pending = (h, qb, ou, den128)
                    if h == 1:
                        pending_oproj = qb
            emit_norm(pending)
            emit_oproj_quarter(pending_oproj)

    nc.finalize()
    return nc


def _get_nc(causal: bool):
    key = bool(causal)
    if key not in _BUILD_CACHE:
        _BUILD_CACHE[key] = _build(causal)
    return _BUILD_CACHE[key]


def _rope_tables(position_ids_b):
    # cosT/sinT: [HD, S] bf16, transposed layout for the [d, s] dataflow
    pos = np.asarray(position_ids_b, dtype=np.float64)
    inv = 1.0 / (ROPE_BASE ** (np.arange(0, HD, 2, dtype=np.float64) / HD))
    f = pos[:, None] * inv[None, :]            # [S, HD/2]
    emb = np.concatenate([f, f], axis=1)       # [S, HD]
    cosT = np.ascontiguousarray(np.cos(emb).T).astype(ml_dtypes.bfloat16)
    sinT = np.ascontiguousarray(np.sin(emb).T).astype(ml_dtypes.bfloat16)
    return cosT, sinT


def _is_causal(attention_mask):
    m = np.asarray(attention_mask)
    if m.shape != (B, 1, S, S):
        return False
    tri = np.tril(np.ones((S, S), dtype=bool))
    canon = np.where(tri, np.float32(0.0), np.float32(-1e9))
    return all(np.array_equal(m[b, 0], canon) for b in range(B))


_ONESB_NP = np.ones((P, P), dtype=ml_dtypes.bfloat16)
_SEL4_NP = np.zeros((P, P), dtype=ml_dtypes.bfloat16)
_SEL4_NP[0, :] = 1.0


def _stair():
    # multiplicative staircase: stair[p, j] = 1 if (j - 512) >= p else 0
    j = np.arange(2 * SBLK)[None, :] - SBLK
    p = np.arange(P)[:, None]
    return np.where(j >= p, 1.0, 0.0).astype(ml_dtypes.bfloat16)


def kernel(hidden_state, attention_mask, position_ids, Wq, Wk, Wv, Wo,
           _trace=False, _tmpdir=None):
    global LAST_EXEC_TIME_NS
    hidden_state = np.asarray(hidden_state, dtype=np.float32)
    Wq = np.asarray(Wq, dtype=np.float32)
    Wk = np.asarray(Wk, dtype=np.float32)
    Wv = np.asarray(Wv, dtype=np.float32)
    Wo = np.asarray(Wo, dtype=np.float32)

    causal = _is_causal(attention_mask)
    nc = _get_nc(causal)

    stair = _stair() if causal else None
    _WKP = []
    for W in (Wk, Wv):
        wb = W.astype(ml_dtypes.bfloat16)
        _WKP.append(np.ascontiguousarray(wb.reshape(DCH, P, HD).transpose(1, 0, 2)))
    per_batch = {}
    for b in range(B):
        hTb = hidden_state[b].T.astype(ml_dtypes.bfloat16)
        tiles = hTb.reshape(DCH, P, S)
        hTp0 = np.ascontiguousarray(tiles[:, :, :SBLK].transpose(1, 0, 2))
        hTp1 = np.ascontiguousarray(tiles[:, :, SBLK:].transpose(1, 0, 2))
        cosTb, sinTb = _rope_tables(position_ids[b])
        mb = None
        if not causal:
            mb = np.ascontiguousarray(
                np.exp(np.asarray(attention_mask, dtype=np.float64)[b, 0].T)
            ).astype(ml_dtypes.bfloat16)
        per_batch[b] = (hTp0, hTp1, cosTb, sinTb, mb)

    in_maps = []
    for core in range(8):
        b = core // 4
        hp = core % 4
        hTp0, hTp1, cosTb, sinTb, mb = per_batch[b]
        wq_s = Wq[:, hp * DQ:(hp + 1) * DQ].astype(ml_dtypes.bfloat16)
        im = {
            "hTp0": hTp0,
            "hTp1": hTp1,
            "onesb": _ONESB_NP,
            "sel4": _SEL4_NP,
            "wqp": np.ascontiguousarray(
                wq_s.reshape(DCH, P, DQ).transpose(1, 0, 2)),
            "wkp": _WKP[0],
            "wvp": _WKP[1],
            "wo": np.ascontiguousarray(
                Wo[hp * DQ:(hp + 1) * DQ, :]).astype(ml_dtypes.bfloat16),
            "cosT": cosTb,
            "sinT": sinTb,
        }
        if causal:
            im["stair"] = stair
        else:
            im["emaskT"] = mb
        in_maps.append(im)

    res = run_bass_kernel_spmd(nc, in_maps, core_ids=list(range(8)),
                               trace=_trace, tmpdir=_tmpdir)
    LAST_EXEC_TIME_NS = res.exec_time_ns

    out = np.empty((B, S, D), dtype=np.float32)
    for b in range(B):
        acc = res.results[4 * b]["out_partial"].astype(np.float32).copy()
        for hp in range(1, 4):
            acc += res.results[4 * b + hp]["out_partial"]
        out[b] = acc
    return out
